# revision 18
# baseline (speedup 1.0000x reference)
"""Trainium2 Bass kernel for nn_CapsGATattentionGRU (B=128, T=32, D=32, H=64, F=2048).

Sharding: GRU recurrence replicated on 8 cores (fp16 Whh SBUF-resident,
col-tiled packed matmuls, DVE block-transpose feedback); x-side gate inputs
N-sharded + chunk-wise AllGathered (overlapped with compute); downstream
batch-sharded via one-hot gather matmul.
"""
import os, sys, time
sys.path.insert(0, '/opt/trn_rl_repo')
import numpy as np

import concourse.bass as bass
import concourse.bacc as bacc
import concourse.tile as tile
from concourse import mybir
from concourse.bass_utils import run_bass_kernel_spmd

f16 = mybir.dt.float16
f32 = mybir.dt.float32
AF = mybir.ActivationFunctionType

D, T, H_, B = 32, 32, 64, 128
F = D * H_
KT = 16
NC = 8
SH = B // NC
CH = 8            # steps per AllGather chunk
NCH = B // CH     # number of chunks
DEBUG = os.environ.get("KBUILD_DEBUG", "") == "1"
NOLDW = os.environ.get("KBUILD_NOLDW", "1") == "1"  # dedupe LDWEIGHTS on 2nd matmul/strip


def hd_perm():
    out = np.zeros(2048, np.int64)
    for Hh in range(2):
        for j in range(4):
            hds = Hh*1024 + (np.arange(8)[:, None]*128 + j*32 + np.arange(32)[None, :]).reshape(-1)
            out[(Hh*4+j)*256:(Hh*4+j)*256+256] = hds
    return out

PERM = hd_perm()


def _gate_cols(whmat):
    """whmat (6144, K) -> (K, 6144) transposed with perm'd col order."""
    K = whmat.shape[1]
    out = np.zeros((K, 6144), np.float32)
    for q in range(8):
        hds = PERM[q*256:(q+1)*256]
        for c in range(3):
            out[:, q*768 + c*256: q*768 + (c+1)*256] = whmat[c*2048 + hds].T
    return out


def build_program():
    nc = bacc.Bacc("TRN2", target_bir_lowering=False, debug=False, num_devices=NC)

    xT_d = nc.dram_tensor("xT", [B, 128, KT*32], f16, kind="ExternalInput")
    wih0_d = nc.dram_tensor("wih0", [KT, 128, 768], f16, kind="ExternalInput")
    wih1_d = nc.dram_tensor("wih1", [KT, 128, 768], f16, kind="ExternalInput")
    b0_d = nc.dram_tensor("b0", [1, 768], f16, kind="ExternalInput")
    b1_d = nc.dram_tensor("b1", [1, 768], f16, kind="ExternalInput")
    whh0_d = nc.dram_tensor("whh0", [KT, 128, 6144], f16, kind="ExternalInput")
    whh1_d = nc.dram_tensor("whh1", [KT, 128, 6144], f16, kind="ExternalInput")
    bhn0_d = nc.dram_tensor("bhn0", [128, 2, 256], f16, kind="ExternalInput")
    bhn1_d = nc.dram_tensor("bhn1", [128, 2, 256], f16, kind="ExternalInput")
    sel_d = nc.dram_tensor("sel", [128, SH], f16, kind="ExternalInput")
    awt_d = nc.dram_tensor("awt", [128, 32], f16, kind="ExternalInput")
    ab_d = nc.dram_tensor("ab", [128, 1], f32, kind="ExternalInput")
    od_d = nc.dram_tensor("od", [128, 4], f16, kind="ExternalInput")
    od2_d = nc.dram_tensor("od2", [4, 128], f16, kind="ExternalInput")
    gw_d = nc.dram_tensor("gw", [65, 4, 64], f16, kind="ExternalInput")
    gatt_d = nc.dram_tensor("gat_att", [2, 128, 64], f16, kind="ExternalInput")
    gbias_d = nc.dram_tensor("gat_bias", [2, 128, 64], f16, kind="ExternalInput")
    wc_d = nc.dram_tensor("wc", [16, 128, 128], f16, kind="ExternalInput")
    fw_d = nc.dram_tensor("fw", [65, 32], f16, kind="ExternalInput")
    od2c_d = nc.dram_tensor("od2c", [4, 128, 128], f16, kind="ExternalInput")

    out_d = nc.dram_tensor("out", [32, SH, 32], f16, kind="ExternalOutput")
    if DEBUG:
        dbg_emb = nc.dram_tensor("dbg_emb", [SH, 32, 2048], f16, kind="ExternalOutput")
        dbg_attv = nc.dram_tensor("dbg_attv", [SH, 2048], f16, kind="ExternalOutput")
        dbg_g01 = nc.dram_tensor("dbg_g01", [2, 128, 64], f16, kind="ExternalOutput")
        dbg_caps = nc.dram_tensor("dbg_caps", [16, 128, 16], f16, kind="ExternalOutput")

    ag0_in = nc.dram_tensor("ag0_in", [B, 32, 768], f16)
    ag0_out = nc.dram_tensor("ag0_out", [NCH, NC, CH, 32, 768], f16, addr_space="Shared")
    ag1_in = nc.dram_tensor("ag1_in", [B, 32, 768], f16)
    ag1_out = nc.dram_tensor("ag1_out", [NCH, NC, CH, 32, 768], f16, addr_space="Shared")
    hT0_seq = nc.dram_tensor("hT0_seq", [B, 2, 128, 256], f16)
    hnat = nc.dram_tensor("hnat", [B, 32, 2048], f16)
    emb_mine = nc.dram_tensor("emb_mine", [SH, 32, 2048], f16)
    att_pad = nc.dram_tensor("att_pad", [512, 128], f16)
    fus_nat = nc.dram_tensor("fus_nat", [512, 128], f16)
    caps_pad = nc.dram_tensor("caps_pad", [512, 128], f16)

    def mm_noldw(inst):
        if NOLDW:
            inst.ins.ldweights = False
        return inst

    with tile.TileContext(nc) as tc:
        ctxs = []
        def pool(**kw):
            p = tc.tile_pool(**kw)
            ctxs.append(p)
            return p.__enter__()
        wp = pool(name="wp", bufs=1)
        sb = pool(name="sb", bufs=1)
        gip = pool(name="gip", bufs=2)
        hp = pool(name="hp", bufs=2)
        psp = pool(name="ps", bufs=2, space="PSUM")

        # ---- psum tag rotation: 4 tags x 2 bufs x 1 bank = 8 banks total ----
        ps_ctr = [0]
        def ps_tile(shape, name):
            tag = f"ps{ps_ctr[0] % 4}"
            ps_ctr[0] += 1
            return psp.tile(shape, f32, name=name, tag=tag)

        # one persistent 192KB slot, carved manually (enables region-level deps)
        big_all = wp.tile([128, KT*6144], f16, name="big_all", tag="big")
        def big_tile(name, cols_f16):
            return big_all[:, 0:cols_f16]

        # ================= phase A / C =================
        def phase_x(wih_src, bias_src, ag_in, ag_out, stat_from_x, pfx):
            # slot layout (f16 cols): wih 12288 | xt 4x512 | bias 768 | bbb 768 | gio 2x384x3
            w = big_tile(f"pxw{pfx}", 12288 + 8*512 + 768 + 768 + 6*384)
            wih = w[:, 0:12288].rearrange("p (k n) -> p k n", k=KT)
            for k in range(KT):
                nc.sync.dma_start(wih[:, k, :], wih_src[k])
            xts_ab = [[w[:, 12288+512*(4*ab_+s): 12288+512*(4*ab_+s+1)].rearrange("p (k b) -> p k b", k=KT)
                       for s in range(4)] for ab_ in range(2)]
            bb = w[0:1, 16384:17152]
            nc.sync.dma_start(bb, bias_src[:, :])
            bbb = w[:, 17152:17920]
            ones1 = sb.tile([1, 128], f16, name=f"ones1{pfx}", tag="ones1")
            nc.vector.memset(ones1[:], 1.0)
            for half in range(2):
                pb = ps_tile([128, 384], f"pb{pfx}{half}")
                nc.tensor.matmul(out=pb[:], lhsT=ones1[:], rhs=bb[:, 384*half:384*half+384],
                                 start=True, stop=True)
                nc.vector.tensor_copy(bbb[:, 384*half:384*half+384], pb[:])
            gio_off = 17920
            for p in range(B // 4):
                xts = xts_ab[p % 2]
                for s in range(4):
                    t = p*4 + s
                    if stat_from_x:
                        nc.gpsimd.dma_start(xts[s][:], xT_d[t].rearrange("p (k b) -> p k b", k=KT))
                    else:
                        nc.gpsimd.dma_start(
                            xts[s].rearrange("p (h m) b -> p h m b", h=2),
                            hT0_seq[t].rearrange("h p (m b) -> p h m b", m=8))
                for ch in range(2):
                    ps = ps_tile([128, 384], f"psA{pfx}{p}{ch}")
                    for k in range(KT):
                        for s in range(4):
                            nc.tensor.matmul(
                                out=ps[32*s:32*s+32, :],
                                lhsT=xts[s][:, k, :],
                                rhs=wih[:, k, 384*ch:384*ch+384],
                                start=(k == 0), stop=(k == KT-1),
                                tile_position=(0, 32*s))
                    gio = w[:, gio_off + ((p % 3)*2 + ch)*384: gio_off + ((p % 3)*2 + ch + 1)*384]
                    nc.vector.tensor_add(gio, ps[:], bbb[:, 384*ch:384*ch+384])
                    nc.scalar.dma_start(
                        out=ag_in[p*4:p*4+4, :, 384*ch:384*ch+384].rearrange("s b n -> (s b) n"),
                        in_=gio)
                if p % (CH // 4) == (CH // 4) - 1:
                    c = p // (CH // 4)
                    nc.gpsimd.collective_compute(
                        "AllGather", mybir.AluOpType.bypass,
                        replica_groups=[list(range(NC))],
                        ins=[ag_in[c*CH:(c+1)*CH].opt()],
                        outs=[ag_out[c].opt()])

        # ================= recurrence =================
        def recurrence(whh_src, ag_out, bhn_src, store_hT0, store_hnat, pfx):
            whh = big_all.rearrange("p (k n) -> p k n", k=KT)
            for k in range(KT - 1, -1, -1):
                nc.sync.dma_start(whh[:, k, :], whh_src[k])
            bhn = sb.tile([128, 2, 256], f16, name=f"bhn{pfx}", tag="bhn")
            nc.sync.dma_start(bhn[:], bhn_src[:, :, :])
            hT = [hp.tile([128, 256], f16, name=f"hT{Hh}", tag=f"hT{Hh}", bufs=2) for Hh in range(2)]
            hg = [hp.tile([128, 256], f16, name=f"hg{Hh}", tag=f"hg{Hh}", bufs=2) for Hh in range(2)]
            for Hh in range(2):
                nc.vector.memset(hT[Hh][:], 0.0)
                nc.vector.memset(hg[Hh][:], 0.0)
            for t in range(B):
                newhT = [None, None]
                newhg = [None, None]
                for Hh in range(2):
                    pa = psp.tile([128, 512], f32, name=f"pa{t}{Hh}", tag=f"ps{2*Hh}")
                    pn = psp.tile([128, 256], f32, name=f"pn{t}{Hh}", tag=f"ps{2*Hh+1}")
                    gi_sb = gip.tile([128, 3, 256], f16, name=f"gi{t}_{Hh}", tag="gi")
                    for j in range(4):
                        nc.gpsimd.dma_start(gi_sb[32*j:32*j+32],
                                      ag_out[t // CH, Hh*4+j, t % CH].rearrange("b (c n) -> b c n", c=3))
                    for k in range(KT):
                        lhsT = hT[k // 8][:, 32*(k % 8):32*(k % 8)+32]
                        for j in range(4):
                            base = (Hh*4+j)*768
                            nc.tensor.matmul(out=pa[32*j:32*j+32, :], lhsT=lhsT,
                                rhs=whh[:, k, base:base+512],
                                start=(k == 0), stop=(k == KT-1), tile_position=(0, 32*j))
                        for j in range(4):
                            base = (Hh*4+j)*768
                            mm_noldw(nc.tensor.matmul(out=pn[32*j:32*j+32, :], lhsT=lhsT,
                                rhs=whh[:, k, base+512:base+768],
                                start=(k == 0), stop=(k == KT-1), tile_position=(0, 32*j)))
                    # gates: r/z from pa+gi01; u=1-z and zh=z*hg precomputed off-path
                    ri = sb.tile([128, 512], f16, name=f"ri{t}{Hh}", tag="gt", bufs=1)
                    nc.vector.tensor_add(ri[:], pa[:], gi_sb[:, 0:2, :].rearrange("p c n -> p (c n)"))
                    r = sb.tile([128, 256], f16, name=f"r{t}{Hh}", tag="r", bufs=1)
                    nc.scalar.activation(r[:], ri[:, 0:256], AF.Sigmoid)
                    z = sb.tile([128, 256], f16, name=f"z{t}{Hh}", tag="z", bufs=1)
                    nc.scalar.activation(z[:], ri[:, 256:512], AF.Sigmoid)
                    u = sb.tile([128, 256], f16, name=f"u{t}{Hh}", tag="u", bufs=1)
                    nc.scalar.activation(u[:], z[:], AF.Identity, bias=1.0, scale=-1.0)
                    zh = sb.tile([128, 256], f16, name=f"zh{t}{Hh}", tag="zh", bufs=1)
                    nc.vector.tensor_mul(zh[:], z[:], hg[Hh][:])
                    # n path
                    v1 = sb.tile([128, 256], f16, name=f"v{t}{Hh}", tag="gt2", bufs=2)
                    nc.vector.tensor_add(v1[:], pn[:], bhn[:, Hh, :])
                    tn = sb.tile([128, 256], f16, name=f"tn{t}{Hh}", tag="gt2", bufs=2)
                    nc.vector.tensor_mul(tn[:], r[:], v1[:])
                    tn2 = sb.tile([128, 256], f16, name=f"tn2{t}{Hh}", tag="gt2", bufs=2)
                    nc.vector.tensor_add(tn2[:], tn[:], gi_sb[:, 2, :])
                    n_ = sb.tile([128, 256], f16, name=f"n{t}{Hh}", tag="n", bufs=1)
                    nc.scalar.activation(n_[:], tn2[:], AF.Tanh)
                    un = sb.tile([128, 256], f16, name=f"un{t}{Hh}", tag="un", bufs=1)
                    nc.vector.tensor_mul(un[:], u[:], n_[:])
                    hn = hp.tile([128, 256], f16, name=f"hg{Hh}", tag=f"hg{Hh}")
                    nc.vector.tensor_add(hn[:], un[:], zh[:])
                    newhg[Hh] = hn
                    nhT = hp.tile([128, 256], f16, name=f"hT{Hh}", tag=f"hT{Hh}")
                    nc.vector.transpose(nhT[:], hn[:])
                    newhT[Hh] = nhT
                    if store_hT0:
                        nc.sync.dma_start(out=hT0_seq[t, Hh], in_=nhT[:])
                    if store_hnat:
                        for j in range(4):
                            nc.sync.dma_start(
                                out=hnat[t, :, Hh*1024:(Hh+1)*1024]
                                    .rearrange("b (m j nl) -> b m j nl", m=8, j=4)[:, :, j, :],
                                in_=hn[32*j:32*j+32].rearrange("p (m nl) -> p m nl", m=8))
                hT = newhT
                hg = newhg

        # ================= run pipeline =================
        phase_x(wih0_d, b0_d, ag0_in, ag0_out, True, "a")
        recurrence(whh0_d, ag0_out, bhn0_d, True, False, "0")
        phase_x(wih1_d, b1_d, ag1_in, ag1_out, False, "c")
        recurrence(whh1_d, ag1_out, bhn1_d, False, True, "1")

        # ================= downstream workspace =================
        ds = big_tile("ds", 57344)  # (128, 28x2048) f16 in the big slot
        def R(i, w=2048):
            return ds[:, 2048*i: 2048*i + w]

        # ---- emb gather ----
        selt = sb.tile([128, SH], f16, name="selt", tag="selt")
        nc.sync.dma_start(selt[:], sel_d[:, :])
        hflat = hnat.ap().rearrange("t b f -> t (b f)")
        eflat = emb_mine.ap().rearrange("s b f -> s (b f)")
        for ch in range(16):
            reg = R(2 * (ch % 2), 4096)
            nc.sync.dma_start(reg, hflat[:, 4096*ch:4096*ch+4096])
            emc = R(4 + 2 * (ch % 2), 4096)[0:SH, :]
            for q in range(8):
                pse = ps_tile([SH, 512], f"pse{ch}{q}")
                nc.tensor.matmul(out=pse[:], lhsT=selt[:],
                                 rhs=reg[:, 512*q:512*q+512], start=True, stop=True)
                nc.scalar.activation(emc[:, 512*q:512*q+512], pse[:], AF.Identity)
            nc.scalar.dma_start(out=eflat[:, 4096*ch:4096*ch+4096], in_=emc)
            if DEBUG:
                nc.scalar.dma_start(
                    out=dbg_emb.ap().rearrange("s b f -> s (b f)")[:, 4096*ch:4096*ch+4096],
                    in_=emc)

        # ---- attention ----
        awt = sb.tile([128, 32], f16, name="awt", tag="awt")
        nc.sync.dma_start(awt[:], awt_d[:, :])
        ab = sb.tile([128, 1], f32, name="ab", tag="ab")
        nc.sync.dma_start(ab[:], ab_d[:, :])
        od = sb.tile([128, 4], f16, name="od", tag="od")
        nc.sync.dma_start(od[:], od_d[:, :])
        od2 = sb.tile([4, 128], f16, name="od2", tag="od2")
        nc.sync.dma_start(od2[:], od2_d[:, :])
        vec16 = R(4)[0:16, :]            # (16, 2048) f16
        for g in range(4):
            Ast16 = R(5 + (g % 2))       # tanh(emb) f16 (128, 2048)
            nc.sync.dma_start(R(7), emb_mine[4*g:4*g+4].rearrange("s b f -> (s b) f"))
            nc.scalar.activation(Ast16[:], R(7), AF.Tanh)
            EW = R(8 + (g % 2))
            for q in range(4):
                psaw = ps_tile([128, 512], f"psaw{g}{q}")
                for smp in range(4):
                    nc.tensor.matmul(
                        out=psaw[32*smp:32*smp+32, :],
                        lhsT=awt[32*smp:32*smp+32, :],
                        rhs=Ast16[32*smp:32*smp+32, 512*q:512*q+512],
                        start=True, stop=True, tile_position=(32*smp, 32*smp))
                nc.scalar.activation(EW[:, 512*q:512*q+512], psaw[:], AF.Exp,
                                     bias=ab[:, 0:1], scale=1.0)
                psd = ps_tile([4, 512], f"psd{g}{q}")
                nc.tensor.matmul(out=psd[:], lhsT=od[:], rhs=EW[:, 512*q:512*q+512],
                                 start=True, stop=True)
                rden = R(10)[0:4, 512*q:512*q+512]
                with nc.allow_low_precision(reason="softmax recip fp16 ok"):
                    nc.vector.reciprocal(rden, psd[:])
                V = R(12)[:, 512*q:512*q+512]
                nc.vector.tensor_mul(V, EW[:, 512*q:512*q+512], Ast16[:, 512*q:512*q+512])
                psv = ps_tile([4, 512], f"psv{g}{q}")
                nc.tensor.matmul(out=psv[:], lhsT=od[:], rhs=V, start=True, stop=True)
                vtmp = R(13)[0:4, 512*q:512*q+512]
                nc.vector.tensor_mul(vtmp, psv[:], rden)
                nc.sync.dma_start(out=vec16[4*g:4*g+4, 512*q:512*q+512], in_=vtmp)
        attv = R(13)[0:16, :]
        nc.scalar.activation(attv, vec16, AF.Tanh)
        if DEBUG:
            nc.sync.dma_start(out=dbg_attv[:, :], in_=attv)

        # ---- build xnT (transposed features+ones) and xn_st ----
        zpad = sb.tile([128, 64], f16, name="zpad", tag="zpad")
        nc.vector.memset(zpad[:], 0.0)
        nc.vector.memset(zpad[:, 0:1], 1.0)
        for gg in range(4):
            nc.gpsimd.dma_start(out=att_pad[128*gg:128*gg+128, 64:128],
                              in_=zpad[:])
        for s in range(16):
            nc.gpsimd.dma_start(out=att_pad[32*s:32*s+32, 0:64],
                                in_=attv[s:s+1, :].rearrange("p (d h) -> p d h", d=32))
        xnT = R(14)[:, 0:512]
        nc.sync.dma_start_transpose(xnT, att_pad[:, :])
        xn_st = [R(14)[:, 512 + 64*g: 512 + 64*(g+1)] for g in range(4)]
        for g in range(4):
            for smp in range(4):
                nc.gpsimd.dma_start(out=xn_st[g][32*smp:32*smp+32, :],
                                  in_=attv[4*g+smp:4*g+smp+1, :].rearrange("p (d h) -> p d h", d=32))

        # ---- GAT ----
        gatw = sb.tile([65, 4, 64], f16, name="gatw", tag="gatw")
        nc.sync.dma_start(gatw[:], gw_d[:, :, :])
        gatt = sb.tile([128, 2, 64], f16, name="gatt", tag="gatt")
        nc.sync.dma_start(gatt[:], gatt_d.ap().rearrange("l p h -> p l h"))
        gbias = sb.tile([128, 2, 64], f16, name="gbias", tag="gbias")
        nc.sync.dma_start(gbias[:], gbias_d.ap().rearrange("l p h -> p l h"))

        def gat_layer(L, xT_all, gout_off):
            """xT_all (128, 512) f16 [rows 0:65 = features+ones].
            writes tanh(gat(x)) to R(gout_off)[:, 64g:64g+64] per g."""
            for g in range(4):
                psx = ps_tile([128, 128], f"psx{L}{g}")
                for smp in range(4):
                    bs = 4*g + smp
                    for lr in range(2):
                        nc.tensor.matmul(out=psx[32*smp:32*smp+32, 64*lr:64*lr+64],
                                         lhsT=xT_all[0:65, 32*bs:32*bs+32],
                                         rhs=gatw[:, 2*L+lr, :], start=True, stop=True,
                                         tile_position=(0, 32*smp))
                xl = R(15)[:, 128*g:128*g+64]
                nc.vector.tensor_copy(xl, psx[:, 0:64])
                xr = R(15)[:, 128*g+64:128*g+128]
                nc.vector.tensor_copy(xr, psx[:, 64:128])
                xrf = R(16)[0:4, :]
                for smp in range(4):
                    nc.gpsimd.dma_start(out=xrf[smp:smp+1, :].rearrange("p (d h) -> p d h", d=32),
                                      in_=xr[32*smp:32*smp+32, :])
                e3 = R(17 + g % 2)
                for q in range(4):
                    psxb = ps_tile([128, 512], f"psxb{L}{g}{q}")
                    nc.tensor.matmul(out=psxb[:], lhsT=od2[:], rhs=xrf[:, 512*q:512*q+512],
                                     start=True, stop=True)
                    e1 = R(19)[:, 0:512]
                    nc.vector.tensor_add(
                        e1.rearrange("p (d h) -> p d h", d=8), psxb[:].rearrange("p (d h) -> p d h", d=8),
                        xl[:, None, :].broadcast_to([128, 8, 64]))
                    e2 = R(19)[:, 512:1024]
                    nc.scalar.activation(e2, e1, AF.Lrelu, alpha=0.2)
                    nc.vector.tensor_mul(
                        e3[:, 512*q:512*q+512].rearrange("p (d h) -> p d h", d=8),
                        e2.rearrange("p (d h) -> p d h", d=8),
                        gatt[:, L, :][:, None, :].broadcast_to([128, 8, 64]))
                lg = sb.tile([128, 32], f32, name=f"lg{L}{g}", tag="lg", bufs=1)
                nc.vector.tensor_reduce(lg[:], e3[:].rearrange("p (d h) -> p d h", d=32),
                                        axis=mybir.AxisListType.X, op=mybir.AluOpType.add)
                elg = sb.tile([128, 32], f16, name=f"elg{L}{g}", tag="elg", bufs=1)
                nc.scalar.activation(elg[:], lg[:], AF.Exp)
                psd2 = ps_tile([4, 32], f"psd2{L}{g}")
                nc.tensor.matmul(out=psd2[:], lhsT=od[:], rhs=elg[:], start=True, stop=True)
                rd2 = sb.tile([4, 32], f16, name=f"rd2{L}{g}", tag="rd2", bufs=1)
                with nc.allow_low_precision(reason="softmax recip fp16 ok"):
                    nc.vector.reciprocal(rd2[:], psd2[:])
                psb2 = ps_tile([128, 32], f"psb2{L}{g}")
                nc.tensor.matmul(out=psb2[:], lhsT=od2[:], rhs=rd2[:], start=True, stop=True)
                alp = sb.tile([128, 32], f16, name=f"alp{L}{g}", tag="alp", bufs=1)
                nc.vector.tensor_mul(alp[:], elg[:], psb2[:])
                psg = ps_tile([128, 64], f"psg{L}{g}")
                for smp in range(4):
                    nc.tensor.matmul(out=psg[32*smp:32*smp+32, :],
                                     lhsT=alp[32*smp:32*smp+32, :],
                                     rhs=xl[32*smp:32*smp+32, :],
                                     start=True, stop=True,
                                     tile_position=(32*smp, 32*smp))
                gb = sb.tile([128, 64], f32, name=f"gb{L}{g}", tag="gb", bufs=1)
                nc.vector.tensor_add(gb[:], psg[:], gbias[:, L, :])
                nc.scalar.activation(R(gout_off)[:, 64*g:64*g+64], gb[:], AF.Tanh)

        gat_layer(0, xnT, 20)
        for gg in range(4):
            nc.gpsimd.dma_start(out=att_pad[128*gg:128*gg+128, 64:128], in_=zpad[:])
            nc.gpsimd.dma_start(out=att_pad[128*gg:128*gg+128, 0:64],
                                in_=R(20)[:, 64*gg:64*gg+64])
        g0T = R(21)[:, 0:512]
        nc.sync.dma_start_transpose(g0T, att_pad[:, :])
        gat_layer(1, g0T, 22)
        if DEBUG:
            nc.sync.dma_start(out=dbg_g01[0], in_=R(20)[:, 0:64])
            nc.sync.dma_start(out=dbg_g01[1], in_=R(22)[:, 0:64])

        # ---- fusion ----
        for g in range(4):
            gs = R(21)[:, 512 + 64*g: 512 + 64*(g+1)]
            nc.vector.tensor_add(gs, R(20)[:, 64*g:64*g+64], R(22)[:, 64*g:64*g+64])
            nc.gpsimd.dma_start(out=fus_nat[128*g:128*g+128, 0:64], in_=xn_st[g])
            nc.gpsimd.dma_start(out=fus_nat[128*g:128*g+128, 64:128], in_=gs)
        fusT = R(23)[:, 0:512]
        nc.sync.dma_start_transpose(fusT, fus_nat[:, :])

        # ---- caps (double-buffered workspaces A/B across mt) ----
        fwt = sb.tile([65, 32], f16, name="fwt", tag="fwt")
        nc.sync.dma_start(fwt[:], fw_d[:, :])
        for gg in range(4):
            nc.gpsimd.dma_start(out=caps_pad[128*gg:128*gg+128, 64:128], in_=zpad[:])
        od2c = R(27)[:, 0:512].rearrange("p (m c) -> p m c", m=4)
        nc.sync.dma_start(od2c[:], od2c_d.ap().rearrange("m p c -> p m c"))
        RA = [24, 16, 18, 20]
        RB = [25, 17, 19, 21]
        for mtg in range(4):
            o0s_l = [None]*4
            Lcur_l = [None]*4
            out_l = [None]*4
            for m in range(4):
                mt = 4*mtg + m
                wc = R(26)[:, 128*(mt % 8):128*(mt % 8)+128]
                nc.sync.dma_start(wc, wc_d[mt])
                pscap = ps_tile([128, 512], f"pscap{mt}")
                nc.tensor.matmul(out=pscap[:], lhsT=wc, rhs=fusT, start=True, stop=True)
                P = R(RA[m])[:, 0:512]
                nc.vector.tensor_copy(P, pscap[:])
                o0 = sb.tile([128, 16], f32, name=f"o0{mt}", tag="o0", bufs=2)
                nc.vector.tensor_reduce(o0[:], P.rearrange("p (b c) -> p b c", b=16),
                                        axis=mybir.AxisListType.X, op=mybir.AluOpType.add)
                o0s = R(RB[m])[:, 1536:1552]
                nc.vector.tensor_scalar_mul(o0s, o0[:], 1.0/32.0)
                Lcur = R(RA[m])[:, 512:1024]
                nc.vector.tensor_mul(Lcur.rearrange("p (b c) -> p b c", b=16),
                                     P.rearrange("p (b c) -> p b c", b=16),
                                     o0s[:, :, None].broadcast_to([128, 16, 32]))
                out_l[m] = o0s
                Lcur_l[m] = Lcur
            for it in (1, 2):
                psdC = ps_tile([128, 512], f"psdC{mtg}{it}")
                nc.vector.memset(psdC[:], 1.0)
                for m in range(4):
                    Et = R(RA[m])[:, 1024:1536]
                    nc.scalar.activation(Et, Lcur_l[m], AF.Exp)
                    nc.tensor.matmul(out=psdC[32*m:32*m+4, :], lhsT=od[:], rhs=Et,
                                     start=True, stop=True, tile_position=(0, 32*m),
                                     skip_group_check=True)
                rdenC = R(22)[:, 0:512]
                with nc.allow_low_precision(reason="softmax recip fp16 ok"):
                    nc.vector.reciprocal(rdenC, psdC[:])
                for m in range(4):
                    mt = 4*mtg + m
                    P = R(RA[m])[:, 0:512]
                    Et = R(RA[m])[:, 1024:1536]
                    psbc = ps_tile([128, 512], f"psbc{mt}{it}")
                    nc.tensor.matmul(out=psbc[:], lhsT=od2c[:, m, :], rhs=rdenC,
                                     start=True, stop=True)
                    pt = R(RA[m])[:, 1536:2048]
                    nc.vector.tensor_mul(pt, Et, psbc[:])
                    pp = R(RB[m])[:, 0:512]
                    nc.vector.tensor_mul(pp, pt, P)
                    oo = sb.tile([128, 16], f32, name=f"oo{mt}{it}", tag="o0", bufs=2)
                    nc.vector.tensor_reduce(oo[:], pp.rearrange("p (b c) -> p b c", b=16),
                                            axis=mybir.AxisListType.X, op=mybir.AluOpType.add)
                    oos = R(RB[m])[:, 1552 + 16*it: 1568 + 16*it]
                    nc.vector.tensor_copy(oos, oo[:])
                    out_l[m] = oos
                    if it == 1:
                        m2 = R(RB[m])[:, 512:1024]
                        nc.vector.tensor_mul(m2.rearrange("p (b c) -> p b c", b=16),
                                             P.rearrange("p (b c) -> p b c", b=16),
                                             oos[:, :, None].broadcast_to([128, 16, 32]))
                        L2 = R(RB[m])[:, 1024:1536]
                        nc.vector.tensor_add(L2, Lcur_l[m], m2)
                        Lcur_l[m] = L2
            for m in range(4):
                mt = 4*mtg + m
                tc_t = R(RB[m])[:, 1600:1616]
                nc.scalar.activation(tc_t, out_l[m], AF.Tanh)
                if DEBUG:
                    nc.sync.dma_start(out=dbg_caps[mt], in_=tc_t)
                for l_loc in range(4):
                    nc.gpsimd.dma_start(
                        out=caps_pad[:, 4*mt+l_loc].rearrange("(s o) -> o s", s=16),
                        in_=tc_t[32*l_loc:32*l_loc+32, :])
        capsT = R(23)[:, 512:1024]
        nc.sync.dma_start_transpose(capsT, caps_pad[:, :])
        psf = ps_tile([32, 512], "psf")
        nc.tensor.matmul(out=psf[:], lhsT=fwt[:], rhs=capsT[0:65, :], start=True, stop=True)
        fin = R(25)[0:32, 1024:1536]
        nc.scalar.activation(fin, psf[:], AF.Tanh)
        nc.sync.dma_start(out=out_d.ap().rearrange("dd s o -> dd (s o)"),
                          in_=fin)

        for p_ in reversed(ctxs):
            p_.__exit__(None, None, None)
    nc.compile()
    return nc


# ===================== host side =====================
_NC_CACHE = {}

def _get_program():
    if "prog" not in _NC_CACHE:
        _NC_CACHE["prog"] = build_program()
    return _NC_CACHE["prog"]


def _prep_inputs(inputs):
    X = np.asarray(inputs["inputs"], np.float32)
    X = np.nan_to_num(X, nan=0.0, posinf=1.0)
    ei = np.asarray(inputs["edge_index"])
    s = np.repeat(np.arange(D), D); t = np.tile(np.arange(D), D)
    off = (np.arange(B) * D)[:, None]
    exp_ei = np.stack([(s[None] + off).reshape(-1), (t[None] + off).reshape(-1)]).astype(ei.dtype)
    assert np.array_equal(ei, exp_ei), "edge_index mismatch vs block-diagonal pattern"

    # [B, 128, KT*32]: xT2[b, p, k*32+c] = X[b, c, k*128+p]
    xT = np.ascontiguousarray(
        np.swapaxes(X, 1, 2).reshape(B, KT, 128, 32).transpose(0, 2, 1, 3)
        .reshape(B, 128, KT*32)).astype(np.float16)

    wih0p = _gate_cols(np.asarray(inputs["Wih0"], np.float32))
    wih1p = _gate_cols(np.asarray(inputs["Wih1"], np.float32))
    whh0p = _gate_cols(np.asarray(inputs["Whh0"], np.float32))
    whh1p = _gate_cols(np.asarray(inputs["Whh1"], np.float32))
    whh0_dev = np.ascontiguousarray(whh0p.reshape(KT, 128, 6144)).astype(np.float16)
    whh1_dev = np.ascontiguousarray(whh1p.reshape(KT, 128, 6144)).astype(np.float16)

    def bias_strip(bih, bhh):
        b = np.zeros(6144, np.float32)
        for q in range(8):
            hds = PERM[q*256:(q+1)*256]
            b[q*768+0*256: q*768+1*256] = bih[0*2048 + hds] + bhh[0*2048 + hds]
            b[q*768+1*256: q*768+2*256] = bih[1*2048 + hds] + bhh[1*2048 + hds]
            b[q*768+2*256: q*768+3*256] = bih[2*2048 + hds]
        return b
    bih0 = np.asarray(inputs["bih0"], np.float32); bhh0 = np.asarray(inputs["bhh0"], np.float32)
    bih1 = np.asarray(inputs["bih1"], np.float32); bhh1 = np.asarray(inputs["bhh1"], np.float32)
    bs0 = bias_strip(bih0, bhh0).astype(np.float16)
    bs1 = bias_strip(bih1, bhh1).astype(np.float16)

    def bhn_bcast(bhh):
        outb = np.zeros((128, 2, 256), np.float32)
        for Hh in range(2):
            for j in range(4):
                hds = PERM[(Hh*4+j)*256:(Hh*4+j)*256+256]
                outb[32*j:32*j+32, Hh, :] = bhh[2*2048 + hds][None, :]
        return outb.astype(np.float16)

    A_w = np.asarray(inputs["A_w"], np.float32); A_b = np.asarray(inputs["A_b"], np.float32)
    awt = np.tile(A_w.T.astype(np.float16), (4, 1))
    ab = np.tile(A_b, 4)[:, None].astype(np.float32)
    od = np.zeros((128, 4), np.float16)
    for gq in range(4):
        od[32*gq:32*gq+32, gq] = 1.0
    od2 = np.ascontiguousarray(od.T)

    gw = np.zeros((65, 4, 64), np.float16)
    for L, pfx in enumerate(["g0", "g1"]):
        for lr, nm in enumerate(["l", "r"]):
            gw[0:64, 2*L+lr] = np.asarray(inputs[f"{pfx}_W{nm}"], np.float32).T.astype(np.float16)
            gw[64, 2*L+lr] = np.asarray(inputs[f"{pfx}_b{nm}"], np.float32).astype(np.float16)
    gat_att = np.zeros((2, 128, 64), np.float16)
    gat_bias = np.zeros((2, 128, 64), np.float16)
    for L, pfx in enumerate(["g0", "g1"]):
        gat_att[L] = np.tile(np.asarray(inputs[f"{pfx}_att"], np.float32), (128, 1)).astype(np.float16)
        gat_bias[L] = np.tile(np.asarray(inputs[f"{pfx}_bias"], np.float32), (128, 1)).astype(np.float16)

    Wc = np.asarray(inputs["W_caps"], np.float32)
    wc_t = np.zeros((16, 128, 128), np.float16)
    for mt in range(16):
        for l_loc in range(4):
            l = 4*mt + l_loc
            wc_t[mt, :, 32*l_loc:32*l_loc+32] = Wc[:, l, :].T.astype(np.float16)
    od2c = np.zeros((4, 128, 128), np.float16)
    for m in range(4):
        for g in range(4):
            for t_ in range(32):
                od2c[m, 32*m+g, 32*g+t_] = 1.0
    fw = np.zeros((65, 32), np.float16)
    fw[0:64] = np.asarray(inputs["F_w"], np.float32).T.astype(np.float16)
    fw[64] = np.asarray(inputs["F_b"], np.float32).astype(np.float16)

    common = dict(xT=xT, whh0=whh0_dev, whh1=whh1_dev,
                  bhn0=bhn_bcast(bhh0), bhn1=bhn_bcast(bhh1),
                  awt=awt, ab=ab, od=od, od2=od2, gw=gw, gat_att=gat_att,
                  gat_bias=gat_bias, wc=wc_t, fw=fw, od2c=od2c)
    in_maps = []
    for r in range(NC):
        sel = np.zeros((128, SH), np.float16)
        for i in range(SH):
            sel[SH*r + i, i] = 1.0
        m = dict(common)
        m["wih0"] = np.ascontiguousarray(wih0p[:, 768*r:768*r+768].astype(np.float16).reshape(KT, 128, 768))
        m["wih1"] = np.ascontiguousarray(wih1p[:, 768*r:768*r+768].astype(np.float16).reshape(KT, 128, 768))
        m["b0"] = bs0[768*r:768*r+768][None, :].copy()
        m["b1"] = bs1[768*r:768*r+768][None, :].copy()
        m["sel"] = sel
        in_maps.append(m)
    return in_maps


def kernel(**inputs):
    in_maps = _prep_inputs(inputs)
    nc = _get_program()
    res = run_bass_kernel_spmd(nc, in_maps, list(range(NC)))
    out = np.concatenate([res.results[r]["out"].transpose(1, 2, 0) for r in range(NC)], axis=0)
    return out.astype(np.float32)


if __name__ == "__main__":
    t0 = time.time()
    build_program()
    print("build+compile", time.time() - t0)


# revision 20
# speedup vs baseline: 1.0071x; 1.0071x over previous
"""Trainium2 Bass kernel for nn_CapsGATattentionGRU (B=128, T=32, D=32, H=64, F=2048).

Sharding: GRU recurrence replicated on 8 cores (fp16 Whh SBUF-resident,
col-tiled packed matmuls, DVE block-transpose feedback); x-side gate inputs
N-sharded + chunk-wise AllGathered (overlapped with compute); downstream
batch-sharded via one-hot gather matmul.
"""
import os, sys, time
sys.path.insert(0, '/opt/trn_rl_repo')
import numpy as np

import concourse.bass as bass
import concourse.bacc as bacc
import concourse.tile as tile
from concourse import mybir
from concourse.bass_utils import run_bass_kernel_spmd

f16 = mybir.dt.float16
f32 = mybir.dt.float32
AF = mybir.ActivationFunctionType

D, T, H_, B = 32, 32, 64, 128
F = D * H_
KT = 16
NC = 8
SH = B // NC
CH = 8            # steps per AllGather chunk
NCH = B // CH     # number of chunks
DEBUG = os.environ.get("KBUILD_DEBUG", "") == "1"
NOLDW = os.environ.get("KBUILD_NOLDW", "1") == "1"  # dedupe LDWEIGHTS on 2nd matmul/strip


def hd_perm():
    out = np.zeros(2048, np.int64)
    for Hh in range(2):
        for j in range(4):
            hds = Hh*1024 + (np.arange(8)[:, None]*128 + j*32 + np.arange(32)[None, :]).reshape(-1)
            out[(Hh*4+j)*256:(Hh*4+j)*256+256] = hds
    return out

PERM = hd_perm()


def _gate_cols(whmat):
    """whmat (6144, K) -> (K, 6144) transposed with perm'd col order."""
    K = whmat.shape[1]
    out = np.zeros((K, 6144), np.float32)
    for q in range(8):
        hds = PERM[q*256:(q+1)*256]
        for c in range(3):
            out[:, q*768 + c*256: q*768 + (c+1)*256] = whmat[c*2048 + hds].T
    return out


def build_program():
    nc = bacc.Bacc("TRN2", target_bir_lowering=False, debug=False, num_devices=NC)

    xT_d = nc.dram_tensor("xT", [B, 128, KT*32], f16, kind="ExternalInput")
    wih0_d = nc.dram_tensor("wih0", [KT, 128, 768], f16, kind="ExternalInput")
    wih1_d = nc.dram_tensor("wih1", [KT, 128, 768], f16, kind="ExternalInput")
    b0_d = nc.dram_tensor("b0", [1, 768], f16, kind="ExternalInput")
    b1_d = nc.dram_tensor("b1", [1, 768], f16, kind="ExternalInput")
    whh0_d = nc.dram_tensor("whh0", [KT, 128, 6144], f16, kind="ExternalInput")
    whh1_d = nc.dram_tensor("whh1", [KT, 128, 6144], f16, kind="ExternalInput")
    bhn0_d = nc.dram_tensor("bhn0", [128, 2, 256], f16, kind="ExternalInput")
    bhn1_d = nc.dram_tensor("bhn1", [128, 2, 256], f16, kind="ExternalInput")
    sel_d = nc.dram_tensor("sel", [128, SH], f16, kind="ExternalInput")
    awt_d = nc.dram_tensor("awt", [128, 32], f16, kind="ExternalInput")
    ab_d = nc.dram_tensor("ab", [128, 1], f32, kind="ExternalInput")
    od_d = nc.dram_tensor("od", [128, 4], f16, kind="ExternalInput")
    od2_d = nc.dram_tensor("od2", [4, 128], f16, kind="ExternalInput")
    gw_d = nc.dram_tensor("gw", [65, 4, 64], f16, kind="ExternalInput")
    gatt_d = nc.dram_tensor("gat_att", [2, 128, 64], f16, kind="ExternalInput")
    gbias_d = nc.dram_tensor("gat_bias", [2, 128, 64], f16, kind="ExternalInput")
    wc_d = nc.dram_tensor("wc", [16, 128, 128], f16, kind="ExternalInput")
    fw_d = nc.dram_tensor("fw", [65, 32], f16, kind="ExternalInput")
    od2c_d = nc.dram_tensor("od2c", [4, 128, 128], f16, kind="ExternalInput")

    out_d = nc.dram_tensor("out", [32, SH, 32], f16, kind="ExternalOutput")
    if DEBUG:
        dbg_emb = nc.dram_tensor("dbg_emb", [SH, 32, 2048], f16, kind="ExternalOutput")
        dbg_attv = nc.dram_tensor("dbg_attv", [SH, 2048], f16, kind="ExternalOutput")
        dbg_g01 = nc.dram_tensor("dbg_g01", [2, 128, 64], f16, kind="ExternalOutput")
        dbg_caps = nc.dram_tensor("dbg_caps", [16, 128, 16], f16, kind="ExternalOutput")

    ag0_in = nc.dram_tensor("ag0_in", [B, 32, 768], f16)
    ag0_out = nc.dram_tensor("ag0_out", [NCH, NC, CH, 32, 768], f16, addr_space="Shared")
    ag1_in = nc.dram_tensor("ag1_in", [B, 32, 768], f16)
    ag1_out = nc.dram_tensor("ag1_out", [NCH, NC, CH, 32, 768], f16, addr_space="Shared")
    hT0_seq = nc.dram_tensor("hT0_seq", [B, 2, 128, 256], f16)
    hnat = nc.dram_tensor("hnat", [B, 32, 2048], f16)
    emb_mine = nc.dram_tensor("emb_mine", [SH, 32, 2048], f16)
    att_pad = nc.dram_tensor("att_pad", [512, 128], f16)
    fus_nat = nc.dram_tensor("fus_nat", [512, 128], f16)
    caps_pad = nc.dram_tensor("caps_pad", [512, 128], f16)

    def mm_noldw(inst):
        if NOLDW:
            inst.ins.ldweights = False
        return inst

    with tile.TileContext(nc) as tc:
        ctxs = []
        def pool(**kw):
            p = tc.tile_pool(**kw)
            ctxs.append(p)
            return p.__enter__()
        wp = pool(name="wp", bufs=1)
        sb = pool(name="sb", bufs=1)
        gip = pool(name="gip", bufs=2)
        hp = pool(name="hp", bufs=2)
        psp = pool(name="ps", bufs=2, space="PSUM")

        # ---- psum tag rotation: 4 tags x 2 bufs x 1 bank = 8 banks total ----
        ps_ctr = [0]
        def ps_tile(shape, name):
            tag = f"ps{ps_ctr[0] % 4}"
            ps_ctr[0] += 1
            return psp.tile(shape, f32, name=name, tag=tag)

        # one persistent 192KB slot, carved manually (enables region-level deps)
        big_all = wp.tile([128, KT*6144], f16, name="big_all", tag="big")
        def big_tile(name, cols_f16):
            return big_all[:, 0:cols_f16]

        # ================= phase A / C =================
        def phase_x(wih_src, bias_src, ag_in, ag_out, stat_from_x, pfx):
            # slot layout (f16 cols): wih 12288 | xt 4x512 | bias 768 | bbb 768 | gio 2x384x3
            w = big_tile(f"pxw{pfx}", 12288 + 8*512 + 768 + 768 + 6*384)
            wih = w[:, 0:12288].rearrange("p (k n) -> p k n", k=KT)
            for k in range(KT):
                nc.sync.dma_start(wih[:, k, :], wih_src[k])
            xts_ab = [[w[:, 12288+512*(4*ab_+s): 12288+512*(4*ab_+s+1)].rearrange("p (k b) -> p k b", k=KT)
                       for s in range(4)] for ab_ in range(2)]
            bb = w[0:1, 16384:17152]
            nc.sync.dma_start(bb, bias_src[:, :])
            bbb = w[:, 17152:17920]
            ones1 = sb.tile([1, 128], f16, name=f"ones1{pfx}", tag="ones1")
            nc.vector.memset(ones1[:], 1.0)
            for half in range(2):
                pb = ps_tile([128, 384], f"pb{pfx}{half}")
                nc.tensor.matmul(out=pb[:], lhsT=ones1[:], rhs=bb[:, 384*half:384*half+384],
                                 start=True, stop=True)
                nc.vector.tensor_copy(bbb[:, 384*half:384*half+384], pb[:])
            gio_off = 17920
            for p in range(B // 4):
                xts = xts_ab[p % 2]
                for s in range(4):
                    t = p*4 + s
                    if stat_from_x:
                        nc.gpsimd.dma_start(xts[s][:], xT_d[t].rearrange("p (k b) -> p k b", k=KT))
                    else:
                        nc.gpsimd.dma_start(
                            xts[s].rearrange("p (h m) b -> p h m b", h=2),
                            hT0_seq[t].rearrange("h p (m b) -> p h m b", m=8))
                for ch in range(2):
                    ps = ps_tile([128, 384], f"psA{pfx}{p}{ch}")
                    for k in range(KT):
                        for s in range(4):
                            nc.tensor.matmul(
                                out=ps[32*s:32*s+32, :],
                                lhsT=xts[s][:, k, :],
                                rhs=wih[:, k, 384*ch:384*ch+384],
                                start=(k == 0), stop=(k == KT-1),
                                tile_position=(0, 32*s))
                    gio = w[:, gio_off + ((p % 3)*2 + ch)*384: gio_off + ((p % 3)*2 + ch + 1)*384]
                    nc.vector.tensor_add(gio, ps[:], bbb[:, 384*ch:384*ch+384])
                    nc.scalar.dma_start(
                        out=ag_in[p*4:p*4+4, :, 384*ch:384*ch+384].rearrange("s b n -> (s b) n"),
                        in_=gio)
                if p % (CH // 4) == (CH // 4) - 1:
                    c = p // (CH // 4)
                    nc.gpsimd.collective_compute(
                        "AllGather", mybir.AluOpType.bypass,
                        replica_groups=[list(range(NC))],
                        ins=[ag_in[c*CH:(c+1)*CH].opt()],
                        outs=[ag_out[c].opt()])

        # ================= recurrence =================
        def recurrence(whh_src, ag_out, bhn_src, store_hT0, store_hnat, pfx):
            whh = big_all.rearrange("p (k n) -> p k n", k=KT)
            for k in range(KT - 1, -1, -1):
                nc.sync.dma_start(whh[:, k, :], whh_src[k])
            bhn = sb.tile([128, 2, 256], f16, name=f"bhn{pfx}", tag="bhn")
            nc.sync.dma_start(bhn[:], bhn_src[:, :, :])
            hT = [hp.tile([128, 256], f16, name=f"hT{Hh}", tag=f"hT{Hh}", bufs=2) for Hh in range(2)]
            hg = [hp.tile([128, 256], f16, name=f"hg{Hh}", tag=f"hg{Hh}", bufs=2) for Hh in range(2)]
            for Hh in range(2):
                nc.vector.memset(hT[Hh][:], 0.0)
                nc.vector.memset(hg[Hh][:], 0.0)
            for t in range(B):
                newhT = [None, None]
                newhg = [None, None]
                for Hh in range(2):
                    pa = psp.tile([128, 512], f32, name=f"pa{t}{Hh}", tag=f"ps{2*Hh}")
                    pn = psp.tile([128, 256], f32, name=f"pn{t}{Hh}", tag=f"ps{2*Hh+1}")
                    gi_sb = gip.tile([128, 3, 256], f16, name=f"gi{t}_{Hh}", tag="gi")
                    for j in range(4):
                        nc.gpsimd.dma_start(gi_sb[32*j:32*j+32],
                                      ag_out[t // CH, Hh*4+j, t % CH].rearrange("b (c n) -> b c n", c=3))
                    for k in range(KT):
                        lhsT = hT[k // 8][:, 32*(k % 8):32*(k % 8)+32]
                        for j in range(4):
                            base = (Hh*4+j)*768
                            nc.tensor.matmul(out=pa[32*j:32*j+32, :], lhsT=lhsT,
                                rhs=whh[:, k, base:base+512],
                                start=(k == 0), stop=(k == KT-1), tile_position=(0, 32*j))
                        for j in range(4):
                            base = (Hh*4+j)*768
                            mm_noldw(nc.tensor.matmul(out=pn[32*j:32*j+32, :], lhsT=lhsT,
                                rhs=whh[:, k, base+512:base+768],
                                start=(k == 0), stop=(k == KT-1), tile_position=(0, 32*j)))
                    # gates: r/z from pa+gi01; u=1-z and zh=z*hg precomputed off-path
                    ri = sb.tile([128, 512], f16, name=f"ri{t}{Hh}", tag="gt", bufs=1)
                    nc.vector.tensor_add(ri[:], pa[:], gi_sb[:, 0:2, :].rearrange("p c n -> p (c n)"))
                    r = sb.tile([128, 256], f16, name=f"r{t}{Hh}", tag="r", bufs=1)
                    nc.scalar.activation(r[:], ri[:, 0:256], AF.Sigmoid)
                    z = sb.tile([128, 256], f16, name=f"z{t}{Hh}", tag="z", bufs=1)
                    nc.scalar.activation(z[:], ri[:, 256:512], AF.Sigmoid)
                    u = sb.tile([128, 256], f16, name=f"u{t}{Hh}", tag="u", bufs=1)
                    nc.scalar.activation(u[:], z[:], AF.Identity, bias=1.0, scale=-1.0)
                    zh = sb.tile([128, 256], f16, name=f"zh{t}{Hh}", tag="zh", bufs=1)
                    nc.vector.tensor_mul(zh[:], z[:], hg[Hh][:])
                    # n path
                    v1 = sb.tile([128, 256], f16, name=f"v{t}{Hh}", tag="gt2", bufs=2)
                    nc.vector.tensor_add(v1[:], pn[:], bhn[:, Hh, :])
                    tn = sb.tile([128, 256], f16, name=f"tn{t}{Hh}", tag="gt2", bufs=2)
                    nc.vector.tensor_mul(tn[:], r[:], v1[:])
                    tn2 = sb.tile([128, 256], f16, name=f"tn2{t}{Hh}", tag="gt2", bufs=2)
                    nc.vector.tensor_add(tn2[:], tn[:], gi_sb[:, 2, :])
                    n_ = sb.tile([128, 256], f16, name=f"n{t}{Hh}", tag="n", bufs=1)
                    nc.scalar.activation(n_[:], tn2[:], AF.Tanh)
                    un = sb.tile([128, 256], f16, name=f"un{t}{Hh}", tag="un", bufs=1)
                    nc.vector.tensor_mul(un[:], u[:], n_[:])
                    hn = hp.tile([128, 256], f16, name=f"hg{Hh}", tag=f"hg{Hh}")
                    nc.vector.tensor_add(hn[:], un[:], zh[:])
                    newhg[Hh] = hn
                    nhT = hp.tile([128, 256], f16, name=f"hT{Hh}", tag=f"hT{Hh}")
                    nc.vector.transpose(nhT[:], hn[:])
                    newhT[Hh] = nhT
                    if store_hT0:
                        nc.sync.dma_start(out=hT0_seq[t, Hh], in_=nhT[:])
                    if store_hnat:
                        for j in range(4):
                            nc.sync.dma_start(
                                out=hnat[t, :, Hh*1024:(Hh+1)*1024]
                                    .rearrange("b (m j nl) -> b m j nl", m=8, j=4)[:, :, j, :],
                                in_=hn[32*j:32*j+32].rearrange("p (m nl) -> p m nl", m=8))
                hT = newhT
                hg = newhg

        # ================= run pipeline =================
        phase_x(wih0_d, b0_d, ag0_in, ag0_out, True, "a")
        recurrence(whh0_d, ag0_out, bhn0_d, True, False, "0")
        phase_x(wih1_d, b1_d, ag1_in, ag1_out, False, "c")
        recurrence(whh1_d, ag1_out, bhn1_d, False, True, "1")

        # ================= downstream workspace =================
        ds = big_tile("ds", 57344)  # (128, 28x2048) f16 in the big slot
        def R(i, w=2048):
            return ds[:, 2048*i: 2048*i + w]

        # ---- emb gather ----
        selt = sb.tile([128, SH], f16, name="selt", tag="selt")
        nc.sync.dma_start(selt[:], sel_d[:, :])
        hflat = hnat.ap().rearrange("t b f -> t (b f)")
        eflat = emb_mine.ap().rearrange("s b f -> s (b f)")
        for ch in range(16):
            reg = R(2 * (ch % 2), 4096)
            nc.sync.dma_start(reg, hflat[:, 4096*ch:4096*ch+4096])
            emc = R(4 + 2 * (ch % 2), 4096)[0:SH, :]
            for q in range(8):
                pse = ps_tile([SH, 512], f"pse{ch}{q}")
                nc.tensor.matmul(out=pse[:], lhsT=selt[:],
                                 rhs=reg[:, 512*q:512*q+512], start=True, stop=True)
                nc.scalar.activation(emc[:, 512*q:512*q+512], pse[:], AF.Identity)
            nc.scalar.dma_start(out=eflat[:, 4096*ch:4096*ch+4096], in_=emc)
            if DEBUG:
                nc.scalar.dma_start(
                    out=dbg_emb.ap().rearrange("s b f -> s (b f)")[:, 4096*ch:4096*ch+4096],
                    in_=emc)

        # ---- attention ----
        awt = sb.tile([128, 32], f16, name="awt", tag="awt")
        nc.sync.dma_start(awt[:], awt_d[:, :])
        ab = sb.tile([128, 1], f32, name="ab", tag="ab")
        nc.sync.dma_start(ab[:], ab_d[:, :])
        od = sb.tile([128, 4], f16, name="od", tag="od")
        nc.sync.dma_start(od[:], od_d[:, :])
        od2 = sb.tile([4, 128], f16, name="od2", tag="od2")
        nc.sync.dma_start(od2[:], od2_d[:, :])
        vec16 = R(4)[0:16, :]            # (16, 2048) f16
        for g in range(4):
            Ast16 = R(5 + (g % 2))       # tanh(emb) f16 (128, 2048)
            nc.sync.dma_start(R(7), emb_mine[4*g:4*g+4].rearrange("s b f -> (s b) f"))
            nc.scalar.activation(Ast16[:], R(7), AF.Tanh)
            EW = R(8 + (g % 2))
            psdP = ps_tile([128, 512], f"psdP{g}")
            nc.vector.memset(psdP[:], 1.0)
            psvP = ps_tile([128, 512], f"psvP{g}")
            for q in range(4):
                psaw = ps_tile([128, 512], f"psaw{g}{q}")
                for smp in range(4):
                    nc.tensor.matmul(
                        out=psaw[32*smp:32*smp+32, :],
                        lhsT=awt[32*smp:32*smp+32, :],
                        rhs=Ast16[32*smp:32*smp+32, 512*q:512*q+512],
                        start=True, stop=True, tile_position=(32*smp, 32*smp))
                nc.scalar.activation(EW[:, 512*q:512*q+512], psaw[:], AF.Exp,
                                     bias=ab[:, 0:1], scale=1.0)
                nc.tensor.matmul(out=psdP[32*q:32*q+4, :], lhsT=od[:],
                                 rhs=EW[:, 512*q:512*q+512],
                                 start=True, stop=True, tile_position=(0, 32*q),
                                 skip_group_check=True)
                V = R(12)[:, 512*q:512*q+512]
                nc.vector.tensor_mul(V, EW[:, 512*q:512*q+512], Ast16[:, 512*q:512*q+512])
                nc.tensor.matmul(out=psvP[32*q:32*q+4, :], lhsT=od[:], rhs=V,
                                 start=True, stop=True, tile_position=(0, 32*q),
                                 skip_group_check=True)
            rdenP = R(10)[:, 512*(g%2):512*(g%2)+512]
            with nc.allow_low_precision(reason="softmax recip fp16 ok"):
                nc.vector.reciprocal(rdenP, psdP[:])
            vtmpP = R(13)[:, 512*(g%2):512*(g%2)+512]
            nc.vector.tensor_mul(vtmpP, psvP[:], rdenP)
            for q in range(4):
                nc.sync.dma_start(out=vec16[4*g:4*g+4, 512*q:512*q+512],
                                  in_=vtmpP[32*q:32*q+4, :])
        attv = R(13)[0:16, :]
        nc.scalar.activation(attv, vec16, AF.Tanh)
        if DEBUG:
            nc.sync.dma_start(out=dbg_attv[:, :], in_=attv)

        # ---- build xnT (transposed features+ones) and xn_st ----
        zpad = sb.tile([128, 64], f16, name="zpad", tag="zpad")
        nc.vector.memset(zpad[:], 0.0)
        nc.vector.memset(zpad[:, 0:1], 1.0)
        for gg in range(4):
            nc.gpsimd.dma_start(out=att_pad[128*gg:128*gg+128, 64:128],
                              in_=zpad[:])
        for s in range(16):
            nc.gpsimd.dma_start(out=att_pad[32*s:32*s+32, 0:64],
                                in_=attv[s:s+1, :].rearrange("p (d h) -> p d h", d=32))
        xnT = R(14)[:, 0:512]
        nc.sync.dma_start_transpose(xnT, att_pad[:, :])
        xn_st = [R(14)[:, 512 + 64*g: 512 + 64*(g+1)] for g in range(4)]
        for g in range(4):
            for smp in range(4):
                nc.gpsimd.dma_start(out=xn_st[g][32*smp:32*smp+32, :],
                                  in_=attv[4*g+smp:4*g+smp+1, :].rearrange("p (d h) -> p d h", d=32))

        # ---- GAT ----
        gatw = sb.tile([65, 4, 64], f16, name="gatw", tag="gatw")
        nc.sync.dma_start(gatw[:], gw_d[:, :, :])
        gatt = sb.tile([128, 2, 64], f16, name="gatt", tag="gatt")
        nc.sync.dma_start(gatt[:], gatt_d.ap().rearrange("l p h -> p l h"))
        gbias = sb.tile([128, 2, 64], f16, name="gbias", tag="gbias")
        nc.sync.dma_start(gbias[:], gbias_d.ap().rearrange("l p h -> p l h"))

        def gat_layer(L, xT_all, gout_off):
            """xT_all (128, 512) f16 [rows 0:65 = features+ones].
            writes tanh(gat(x)) to R(gout_off)[:, 64g:64g+64] per g."""
            for g in range(4):
                psx = ps_tile([128, 128], f"psx{L}{g}")
                for smp in range(4):
                    bs = 4*g + smp
                    for lr in range(2):
                        nc.tensor.matmul(out=psx[32*smp:32*smp+32, 64*lr:64*lr+64],
                                         lhsT=xT_all[0:65, 32*bs:32*bs+32],
                                         rhs=gatw[:, 2*L+lr, :], start=True, stop=True,
                                         tile_position=(0, 32*smp))
                xl = R(15)[:, 128*g:128*g+64]
                nc.vector.tensor_copy(xl, psx[:, 0:64])
                xr = R(15)[:, 128*g+64:128*g+128]
                nc.vector.tensor_copy(xr, psx[:, 64:128])
                xrf = R(16)[0:4, :]
                for smp in range(4):
                    nc.gpsimd.dma_start(out=xrf[smp:smp+1, :].rearrange("p (d h) -> p d h", d=32),
                                      in_=xr[32*smp:32*smp+32, :])
                e3 = R(17 + g % 2)
                for q in range(4):
                    psxb = ps_tile([128, 512], f"psxb{L}{g}{q}")
                    nc.tensor.matmul(out=psxb[:], lhsT=od2[:], rhs=xrf[:, 512*q:512*q+512],
                                     start=True, stop=True)
                    e1 = R(19)[:, 0:512]
                    nc.vector.tensor_add(
                        e1.rearrange("p (d h) -> p d h", d=8), psxb[:].rearrange("p (d h) -> p d h", d=8),
                        xl[:, None, :].broadcast_to([128, 8, 64]))
                    e2 = R(19)[:, 512:1024]
                    nc.scalar.activation(e2, e1, AF.Lrelu, alpha=0.2)
                    nc.vector.tensor_mul(
                        e3[:, 512*q:512*q+512].rearrange("p (d h) -> p d h", d=8),
                        e2.rearrange("p (d h) -> p d h", d=8),
                        gatt[:, L, :][:, None, :].broadcast_to([128, 8, 64]))
                lg = sb.tile([128, 32], f32, name=f"lg{L}{g}", tag="lg", bufs=1)
                nc.vector.tensor_reduce(lg[:], e3[:].rearrange("p (d h) -> p d h", d=32),
                                        axis=mybir.AxisListType.X, op=mybir.AluOpType.add)
                elg = sb.tile([128, 32], f16, name=f"elg{L}{g}", tag="elg", bufs=1)
                nc.scalar.activation(elg[:], lg[:], AF.Exp)
                psd2 = ps_tile([4, 32], f"psd2{L}{g}")
                nc.tensor.matmul(out=psd2[:], lhsT=od[:], rhs=elg[:], start=True, stop=True)
                rd2 = sb.tile([4, 32], f16, name=f"rd2{L}{g}", tag="rd2", bufs=1)
                with nc.allow_low_precision(reason="softmax recip fp16 ok"):
                    nc.vector.reciprocal(rd2[:], psd2[:])
                psb2 = ps_tile([128, 32], f"psb2{L}{g}")
                nc.tensor.matmul(out=psb2[:], lhsT=od2[:], rhs=rd2[:], start=True, stop=True)
                alp = sb.tile([128, 32], f16, name=f"alp{L}{g}", tag="alp", bufs=1)
                nc.vector.tensor_mul(alp[:], elg[:], psb2[:])
                psg = ps_tile([128, 64], f"psg{L}{g}")
                for smp in range(4):
                    nc.tensor.matmul(out=psg[32*smp:32*smp+32, :],
                                     lhsT=alp[32*smp:32*smp+32, :],
                                     rhs=xl[32*smp:32*smp+32, :],
                                     start=True, stop=True,
                                     tile_position=(32*smp, 32*smp))
                gb = sb.tile([128, 64], f32, name=f"gb{L}{g}", tag="gb", bufs=1)
                nc.vector.tensor_add(gb[:], psg[:], gbias[:, L, :])
                nc.scalar.activation(R(gout_off)[:, 64*g:64*g+64], gb[:], AF.Tanh)

        gat_layer(0, xnT, 20)
        for gg in range(4):
            nc.gpsimd.dma_start(out=att_pad[128*gg:128*gg+128, 64:128], in_=zpad[:])
            nc.gpsimd.dma_start(out=att_pad[128*gg:128*gg+128, 0:64],
                                in_=R(20)[:, 64*gg:64*gg+64])
        g0T = R(21)[:, 0:512]
        nc.sync.dma_start_transpose(g0T, att_pad[:, :])
        gat_layer(1, g0T, 22)
        if DEBUG:
            nc.sync.dma_start(out=dbg_g01[0], in_=R(20)[:, 0:64])
            nc.sync.dma_start(out=dbg_g01[1], in_=R(22)[:, 0:64])

        # ---- fusion ----
        for g in range(4):
            gs = R(21)[:, 512 + 64*g: 512 + 64*(g+1)]
            nc.vector.tensor_add(gs, R(20)[:, 64*g:64*g+64], R(22)[:, 64*g:64*g+64])
            nc.gpsimd.dma_start(out=fus_nat[128*g:128*g+128, 0:64], in_=xn_st[g])
            nc.gpsimd.dma_start(out=fus_nat[128*g:128*g+128, 64:128], in_=gs)
        fusT = R(23)[:, 0:512]
        nc.sync.dma_start_transpose(fusT, fus_nat[:, :])

        # ---- caps (double-buffered workspaces A/B across mt) ----
        fwt = sb.tile([65, 32], f16, name="fwt", tag="fwt")
        nc.sync.dma_start(fwt[:], fw_d[:, :])
        for gg in range(4):
            nc.gpsimd.dma_start(out=caps_pad[128*gg:128*gg+128, 64:128], in_=zpad[:])
        od2c = R(27)[:, 0:512].rearrange("p (m c) -> p m c", m=4)
        nc.sync.dma_start(od2c[:], od2c_d.ap().rearrange("m p c -> p m c"))
        RA = [24, 16, 18, 20]
        RB = [25, 17, 19, 21]
        for mtg in range(4):
            o0s_l = [None]*4
            Lcur_l = [None]*4
            out_l = [None]*4
            for m in range(4):
                mt = 4*mtg + m
                wc = R(26)[:, 128*(mt % 8):128*(mt % 8)+128]
                nc.sync.dma_start(wc, wc_d[mt])
                pscap = ps_tile([128, 512], f"pscap{mt}")
                nc.tensor.matmul(out=pscap[:], lhsT=wc, rhs=fusT, start=True, stop=True)
                P = R(RA[m])[:, 0:512]
                nc.vector.tensor_copy(P, pscap[:])
                o0 = sb.tile([128, 16], f32, name=f"o0{mt}", tag="o0", bufs=2)
                nc.vector.tensor_reduce(o0[:], P.rearrange("p (b c) -> p b c", b=16),
                                        axis=mybir.AxisListType.X, op=mybir.AluOpType.add)
                o0s = R(RB[m])[:, 1536:1552]
                nc.vector.tensor_scalar_mul(o0s, o0[:], 1.0/32.0)
                Lcur = R(RA[m])[:, 512:1024]
                nc.vector.tensor_mul(Lcur.rearrange("p (b c) -> p b c", b=16),
                                     P.rearrange("p (b c) -> p b c", b=16),
                                     o0s[:, :, None].broadcast_to([128, 16, 32]))
                out_l[m] = o0s
                Lcur_l[m] = Lcur
            for it in (1, 2):
                psdC = ps_tile([128, 512], f"psdC{mtg}{it}")
                nc.vector.memset(psdC[:], 1.0)
                for m in range(4):
                    Et = R(RA[m])[:, 1024:1536]
                    nc.scalar.activation(Et, Lcur_l[m], AF.Exp)
                    nc.tensor.matmul(out=psdC[32*m:32*m+4, :], lhsT=od[:], rhs=Et,
                                     start=True, stop=True, tile_position=(0, 32*m),
                                     skip_group_check=True)
                rdenC = R(22)[:, 0:512]
                with nc.allow_low_precision(reason="softmax recip fp16 ok"):
                    nc.vector.reciprocal(rdenC, psdC[:])
                for m in range(4):
                    mt = 4*mtg + m
                    P = R(RA[m])[:, 0:512]
                    Et = R(RA[m])[:, 1024:1536]
                    psbc = ps_tile([128, 512], f"psbc{mt}{it}")
                    nc.tensor.matmul(out=psbc[:], lhsT=od2c[:, m, :], rhs=rdenC,
                                     start=True, stop=True)
                    pt = R(RA[m])[:, 1536:2048]
                    nc.vector.tensor_mul(pt, Et, psbc[:])
                    pp = R(RB[m])[:, 0:512]
                    nc.vector.tensor_mul(pp, pt, P)
                    oo = sb.tile([128, 16], f32, name=f"oo{mt}{it}", tag="o0", bufs=2)
                    nc.vector.tensor_reduce(oo[:], pp.rearrange("p (b c) -> p b c", b=16),
                                            axis=mybir.AxisListType.X, op=mybir.AluOpType.add)
                    oos = R(RB[m])[:, 1552 + 16*it: 1568 + 16*it]
                    nc.vector.tensor_copy(oos, oo[:])
                    out_l[m] = oos
                    if it == 1:
                        m2 = R(RB[m])[:, 512:1024]
                        nc.vector.tensor_mul(m2.rearrange("p (b c) -> p b c", b=16),
                                             P.rearrange("p (b c) -> p b c", b=16),
                                             oos[:, :, None].broadcast_to([128, 16, 32]))
                        L2 = R(RB[m])[:, 1024:1536]
                        nc.vector.tensor_add(L2, Lcur_l[m], m2)
                        Lcur_l[m] = L2
            for m in range(4):
                mt = 4*mtg + m
                tc_t = R(RB[m])[:, 1600:1616]
                nc.scalar.activation(tc_t, out_l[m], AF.Tanh)
                if DEBUG:
                    nc.sync.dma_start(out=dbg_caps[mt], in_=tc_t)
                for l_loc in range(4):
                    nc.gpsimd.dma_start(
                        out=caps_pad[:, 4*mt+l_loc].rearrange("(s o) -> o s", s=16),
                        in_=tc_t[32*l_loc:32*l_loc+32, :])
        capsT = R(23)[:, 512:1024]
        nc.sync.dma_start_transpose(capsT, caps_pad[:, :])
        psf = ps_tile([32, 512], "psf")
        nc.tensor.matmul(out=psf[:], lhsT=fwt[:], rhs=capsT[0:65, :], start=True, stop=True)
        fin = R(25)[0:32, 1024:1536]
        nc.scalar.activation(fin, psf[:], AF.Tanh)
        nc.sync.dma_start(out=out_d.ap().rearrange("dd s o -> dd (s o)"),
                          in_=fin)

        for p_ in reversed(ctxs):
            p_.__exit__(None, None, None)
    nc.compile()
    return nc


# ===================== host side =====================
_NC_CACHE = {}

def _get_program():
    if "prog" not in _NC_CACHE:
        _NC_CACHE["prog"] = build_program()
    return _NC_CACHE["prog"]


def _prep_inputs(inputs):
    X = np.asarray(inputs["inputs"], np.float32)
    X = np.nan_to_num(X, nan=0.0, posinf=1.0)
    ei = np.asarray(inputs["edge_index"])
    s = np.repeat(np.arange(D), D); t = np.tile(np.arange(D), D)
    off = (np.arange(B) * D)[:, None]
    exp_ei = np.stack([(s[None] + off).reshape(-1), (t[None] + off).reshape(-1)]).astype(ei.dtype)
    assert np.array_equal(ei, exp_ei), "edge_index mismatch vs block-diagonal pattern"

    # [B, 128, KT*32]: xT2[b, p, k*32+c] = X[b, c, k*128+p]
    xT = np.ascontiguousarray(
        np.swapaxes(X, 1, 2).reshape(B, KT, 128, 32).transpose(0, 2, 1, 3)
        .reshape(B, 128, KT*32)).astype(np.float16)

    wih0p = _gate_cols(np.asarray(inputs["Wih0"], np.float32))
    wih1p = _gate_cols(np.asarray(inputs["Wih1"], np.float32))
    whh0p = _gate_cols(np.asarray(inputs["Whh0"], np.float32))
    whh1p = _gate_cols(np.asarray(inputs["Whh1"], np.float32))
    whh0_dev = np.ascontiguousarray(whh0p.reshape(KT, 128, 6144)).astype(np.float16)
    whh1_dev = np.ascontiguousarray(whh1p.reshape(KT, 128, 6144)).astype(np.float16)

    def bias_strip(bih, bhh):
        b = np.zeros(6144, np.float32)
        for q in range(8):
            hds = PERM[q*256:(q+1)*256]
            b[q*768+0*256: q*768+1*256] = bih[0*2048 + hds] + bhh[0*2048 + hds]
            b[q*768+1*256: q*768+2*256] = bih[1*2048 + hds] + bhh[1*2048 + hds]
            b[q*768+2*256: q*768+3*256] = bih[2*2048 + hds]
        return b
    bih0 = np.asarray(inputs["bih0"], np.float32); bhh0 = np.asarray(inputs["bhh0"], np.float32)
    bih1 = np.asarray(inputs["bih1"], np.float32); bhh1 = np.asarray(inputs["bhh1"], np.float32)
    bs0 = bias_strip(bih0, bhh0).astype(np.float16)
    bs1 = bias_strip(bih1, bhh1).astype(np.float16)

    def bhn_bcast(bhh):
        outb = np.zeros((128, 2, 256), np.float32)
        for Hh in range(2):
            for j in range(4):
                hds = PERM[(Hh*4+j)*256:(Hh*4+j)*256+256]
                outb[32*j:32*j+32, Hh, :] = bhh[2*2048 + hds][None, :]
        return outb.astype(np.float16)

    A_w = np.asarray(inputs["A_w"], np.float32); A_b = np.asarray(inputs["A_b"], np.float32)
    awt = np.tile(A_w.T.astype(np.float16), (4, 1))
    ab = np.tile(A_b, 4)[:, None].astype(np.float32)
    od = np.zeros((128, 4), np.float16)
    for gq in range(4):
        od[32*gq:32*gq+32, gq] = 1.0
    od2 = np.ascontiguousarray(od.T)

    gw = np.zeros((65, 4, 64), np.float16)
    for L, pfx in enumerate(["g0", "g1"]):
        for lr, nm in enumerate(["l", "r"]):
            gw[0:64, 2*L+lr] = np.asarray(inputs[f"{pfx}_W{nm}"], np.float32).T.astype(np.float16)
            gw[64, 2*L+lr] = np.asarray(inputs[f"{pfx}_b{nm}"], np.float32).astype(np.float16)
    gat_att = np.zeros((2, 128, 64), np.float16)
    gat_bias = np.zeros((2, 128, 64), np.float16)
    for L, pfx in enumerate(["g0", "g1"]):
        gat_att[L] = np.tile(np.asarray(inputs[f"{pfx}_att"], np.float32), (128, 1)).astype(np.float16)
        gat_bias[L] = np.tile(np.asarray(inputs[f"{pfx}_bias"], np.float32), (128, 1)).astype(np.float16)

    Wc = np.asarray(inputs["W_caps"], np.float32)
    wc_t = np.zeros((16, 128, 128), np.float16)
    for mt in range(16):
        for l_loc in range(4):
            l = 4*mt + l_loc
            wc_t[mt, :, 32*l_loc:32*l_loc+32] = Wc[:, l, :].T.astype(np.float16)
    od2c = np.zeros((4, 128, 128), np.float16)
    for m in range(4):
        for g in range(4):
            for t_ in range(32):
                od2c[m, 32*m+g, 32*g+t_] = 1.0
    fw = np.zeros((65, 32), np.float16)
    fw[0:64] = np.asarray(inputs["F_w"], np.float32).T.astype(np.float16)
    fw[64] = np.asarray(inputs["F_b"], np.float32).astype(np.float16)

    common = dict(xT=xT, whh0=whh0_dev, whh1=whh1_dev,
                  bhn0=bhn_bcast(bhh0), bhn1=bhn_bcast(bhh1),
                  awt=awt, ab=ab, od=od, od2=od2, gw=gw, gat_att=gat_att,
                  gat_bias=gat_bias, wc=wc_t, fw=fw, od2c=od2c)
    in_maps = []
    for r in range(NC):
        sel = np.zeros((128, SH), np.float16)
        for i in range(SH):
            sel[SH*r + i, i] = 1.0
        m = dict(common)
        m["wih0"] = np.ascontiguousarray(wih0p[:, 768*r:768*r+768].astype(np.float16).reshape(KT, 128, 768))
        m["wih1"] = np.ascontiguousarray(wih1p[:, 768*r:768*r+768].astype(np.float16).reshape(KT, 128, 768))
        m["b0"] = bs0[768*r:768*r+768][None, :].copy()
        m["b1"] = bs1[768*r:768*r+768][None, :].copy()
        m["sel"] = sel
        in_maps.append(m)
    return in_maps


def kernel(**inputs):
    in_maps = _prep_inputs(inputs)
    nc = _get_program()
    res = run_bass_kernel_spmd(nc, in_maps, list(range(NC)))
    out = np.concatenate([res.results[r]["out"].transpose(1, 2, 0) for r in range(NC)], axis=0)
    return out.astype(np.float32)


if __name__ == "__main__":
    t0 = time.time()
    build_program()
    print("build+compile", time.time() - t0)


# revision 21
# speedup vs baseline: 1.0272x; 1.0200x over previous
"""Trainium2 Bass kernel for nn_CapsGATattentionGRU (B=128, T=32, D=32, H=64, F=2048).

Sharding: GRU recurrence replicated on 8 cores (fp16 Whh SBUF-resident,
col-tiled packed matmuls, DVE block-transpose feedback); x-side gate inputs
N-sharded + chunk-wise AllGathered (overlapped with compute); downstream
batch-sharded via one-hot gather matmul.
"""
import os, sys, time
sys.path.insert(0, '/opt/trn_rl_repo')
import numpy as np

import concourse.bass as bass
import concourse.bacc as bacc
import concourse.tile as tile
from concourse import mybir
from concourse.bass_utils import run_bass_kernel_spmd

f16 = mybir.dt.float16
f32 = mybir.dt.float32
AF = mybir.ActivationFunctionType

D, T, H_, B = 32, 32, 64, 128
F = D * H_
KT = 16
NC = 8
SH = B // NC
CH = 8            # steps per AllGather chunk
NCH = B // CH     # number of chunks
DEBUG = os.environ.get("KBUILD_DEBUG", "") == "1"
NOLDW = os.environ.get("KBUILD_NOLDW", "1") == "1"  # dedupe LDWEIGHTS on 2nd matmul/strip


def hd_perm():
    out = np.zeros(2048, np.int64)
    for Hh in range(2):
        for j in range(4):
            hds = Hh*1024 + (np.arange(8)[:, None]*128 + j*32 + np.arange(32)[None, :]).reshape(-1)
            out[(Hh*4+j)*256:(Hh*4+j)*256+256] = hds
    return out

PERM = hd_perm()


def _gate_cols(whmat):
    """whmat (6144, K) -> (K, 6144) transposed with perm'd col order."""
    K = whmat.shape[1]
    out = np.zeros((K, 6144), np.float32)
    for q in range(8):
        hds = PERM[q*256:(q+1)*256]
        for c in range(3):
            out[:, q*768 + c*256: q*768 + (c+1)*256] = whmat[c*2048 + hds].T
    return out


def build_program():
    nc = bacc.Bacc("TRN2", target_bir_lowering=False, debug=False, num_devices=NC)

    xT_d = nc.dram_tensor("xT", [B, 128, KT*32], f16, kind="ExternalInput")
    wih0_d = nc.dram_tensor("wih0", [KT, 128, 768], f16, kind="ExternalInput")
    wih1_d = nc.dram_tensor("wih1", [KT, 128, 768], f16, kind="ExternalInput")
    b0_d = nc.dram_tensor("b0", [1, 768], f16, kind="ExternalInput")
    b1_d = nc.dram_tensor("b1", [1, 768], f16, kind="ExternalInput")
    whh0_d = nc.dram_tensor("whh0", [KT, 128, 6144], f16, kind="ExternalInput")
    whh1_d = nc.dram_tensor("whh1", [KT, 128, 6144], f16, kind="ExternalInput")
    bhn0_d = nc.dram_tensor("bhn0", [128, 2, 256], f16, kind="ExternalInput")
    bhn1_d = nc.dram_tensor("bhn1", [128, 2, 256], f16, kind="ExternalInput")
    sel_d = nc.dram_tensor("sel", [128, SH], f16, kind="ExternalInput")
    awt_d = nc.dram_tensor("awt", [128, 32], f16, kind="ExternalInput")
    ab_d = nc.dram_tensor("ab", [128, 1], f32, kind="ExternalInput")
    od_d = nc.dram_tensor("od", [128, 4], f16, kind="ExternalInput")
    od2_d = nc.dram_tensor("od2", [4, 128], f16, kind="ExternalInput")
    gw_d = nc.dram_tensor("gw", [65, 4, 64], f16, kind="ExternalInput")
    gatt_d = nc.dram_tensor("gat_att", [2, 128, 64], f16, kind="ExternalInput")
    gbias_d = nc.dram_tensor("gat_bias", [2, 128, 64], f16, kind="ExternalInput")
    wc_d = nc.dram_tensor("wc", [16, 128, 128], f16, kind="ExternalInput")
    fw_d = nc.dram_tensor("fw", [65, 32], f16, kind="ExternalInput")
    od2c_d = nc.dram_tensor("od2c", [4, 128, 128], f16, kind="ExternalInput")

    out_d = nc.dram_tensor("out", [32, SH, 32], f16, kind="ExternalOutput")
    if DEBUG:
        dbg_emb = nc.dram_tensor("dbg_emb", [SH, 32, 2048], f16, kind="ExternalOutput")
        dbg_attv = nc.dram_tensor("dbg_attv", [SH, 2048], f16, kind="ExternalOutput")
        dbg_g01 = nc.dram_tensor("dbg_g01", [2, 128, 64], f16, kind="ExternalOutput")
        dbg_caps = nc.dram_tensor("dbg_caps", [16, 128, 16], f16, kind="ExternalOutput")

    ag0_in = nc.dram_tensor("ag0_in", [B, 32, 768], f16)
    ag0_out = nc.dram_tensor("ag0_out", [NCH, NC, CH, 32, 768], f16, addr_space="Shared")
    ag1_in = nc.dram_tensor("ag1_in", [B, 32, 768], f16)
    ag1_out = nc.dram_tensor("ag1_out", [NCH, NC, CH, 32, 768], f16, addr_space="Shared")
    hT0_seq = nc.dram_tensor("hT0_seq", [B, 2, 128, 256], f16)
    hnat = nc.dram_tensor("hnat", [B, 32, 2048], f16)
    emb_mine = nc.dram_tensor("emb_mine", [SH, 32, 2048], f16)
    att_pad = nc.dram_tensor("att_pad", [512, 128], f16)
    fus_nat = nc.dram_tensor("fus_nat", [512, 128], f16)
    caps_pad = nc.dram_tensor("caps_pad", [512, 128], f16)

    def mm_noldw(inst):
        if NOLDW:
            inst.ins.ldweights = False
        return inst

    with tile.TileContext(nc) as tc:
        ctxs = []
        def pool(**kw):
            p = tc.tile_pool(**kw)
            ctxs.append(p)
            return p.__enter__()
        wp = pool(name="wp", bufs=1)
        sb = pool(name="sb", bufs=1)
        gip = pool(name="gip", bufs=2)
        hp = pool(name="hp", bufs=2)
        psp = pool(name="ps", bufs=2, space="PSUM")

        # ---- psum tag rotation: 4 tags x 2 bufs x 1 bank = 8 banks total ----
        ps_ctr = [0]
        def ps_tile(shape, name):
            tag = f"ps{ps_ctr[0] % 4}"
            ps_ctr[0] += 1
            return psp.tile(shape, f32, name=name, tag=tag)

        # one persistent 192KB slot, carved manually (enables region-level deps)
        big_all = wp.tile([128, KT*6144], f16, name="big_all", tag="big")
        def big_tile(name, cols_f16):
            return big_all[:, 0:cols_f16]

        # ================= phase A / C =================
        def phase_x(wih_src, bias_src, ag_in, ag_out, stat_from_x, pfx):
            # slot layout (f16 cols): wih 12288 | xt 4x512 | bias 768 | bbb 768 | gio 2x384x3
            w = big_tile(f"pxw{pfx}", 24320)
            wih = w[:, 0:12288].rearrange("p (k n) -> p k n", k=KT)
            for k in range(KT):
                nc.sync.dma_start(wih[:, k, :], wih_src[k])
            xbase = [12288, 14336, 20224, 22272]
            xts_ab = [[w[:, xbase[ab_]+512*s: xbase[ab_]+512*(s+1)].rearrange("p (k b) -> p k b", k=KT)
                       for s in range(4)] for ab_ in range(4)]
            bb = w[0:1, 16384:17152]
            nc.sync.dma_start(bb, bias_src[:, :])
            bbb = w[:, 17152:17920]
            ones1 = sb.tile([1, 128], f16, name=f"ones1{pfx}", tag="ones1")
            nc.vector.memset(ones1[:], 1.0)
            for half in range(2):
                pb = ps_tile([128, 384], f"pb{pfx}{half}")
                nc.tensor.matmul(out=pb[:], lhsT=ones1[:], rhs=bb[:, 384*half:384*half+384],
                                 start=True, stop=True)
                nc.vector.tensor_copy(bbb[:, 384*half:384*half+384], pb[:])
            gio_off = 17920
            for p in range(B // 4):
                xts = xts_ab[p % 4]
                for s in range(4):
                    t = p*4 + s
                    if stat_from_x:
                        nc.gpsimd.dma_start(xts[s][:], xT_d[t].rearrange("p (k b) -> p k b", k=KT))
                    else:
                        nc.gpsimd.dma_start(
                            xts[s].rearrange("p (h m) b -> p h m b", h=2),
                            hT0_seq[t].rearrange("h p (m b) -> p h m b", m=8))
                for ch in range(2):
                    ps = ps_tile([128, 384], f"psA{pfx}{p}{ch}")
                    for k in range(KT):
                        for s in range(4):
                            nc.tensor.matmul(
                                out=ps[32*s:32*s+32, :],
                                lhsT=xts[s][:, k, :],
                                rhs=wih[:, k, 384*ch:384*ch+384],
                                start=(k == 0), stop=(k == KT-1),
                                tile_position=(0, 32*s))
                    gio = w[:, gio_off + ((p % 3)*2 + ch)*384: gio_off + ((p % 3)*2 + ch + 1)*384]
                    nc.vector.tensor_add(gio, ps[:], bbb[:, 384*ch:384*ch+384])
                    nc.scalar.dma_start(
                        out=ag_in[p*4:p*4+4, :, 384*ch:384*ch+384].rearrange("s b n -> (s b) n"),
                        in_=gio)
                if p % (CH // 4) == (CH // 4) - 1:
                    c = p // (CH // 4)
                    nc.gpsimd.collective_compute(
                        "AllGather", mybir.AluOpType.bypass,
                        replica_groups=[list(range(NC))],
                        ins=[ag_in[c*CH:(c+1)*CH].opt()],
                        outs=[ag_out[c].opt()])

        # ================= recurrence =================
        def recurrence(whh_src, ag_out, bhn_src, store_hT0, store_hnat, pfx):
            whh = big_all.rearrange("p (k n) -> p k n", k=KT)
            for k in range(KT - 1, -1, -1):
                nc.sync.dma_start(whh[:, k, :], whh_src[k])
            bhn = sb.tile([128, 2, 256], f16, name=f"bhn{pfx}", tag="bhn")
            nc.sync.dma_start(bhn[:], bhn_src[:, :, :])
            hT = [hp.tile([128, 256], f16, name=f"hT{Hh}", tag=f"hT{Hh}", bufs=2) for Hh in range(2)]
            hg = [hp.tile([128, 256], f16, name=f"hg{Hh}", tag=f"hg{Hh}", bufs=2) for Hh in range(2)]
            for Hh in range(2):
                nc.vector.memset(hT[Hh][:], 0.0)
                nc.vector.memset(hg[Hh][:], 0.0)
            for t in range(B):
                newhT = [None, None]
                newhg = [None, None]
                for Hh in range(2):
                    pa = psp.tile([128, 512], f32, name=f"pa{t}{Hh}", tag=f"ps{2*Hh}")
                    pn = psp.tile([128, 256], f32, name=f"pn{t}{Hh}", tag=f"ps{2*Hh+1}")
                    gi_sb = gip.tile([128, 3, 256], f16, name=f"gi{t}_{Hh}", tag="gi")
                    for j in range(4):
                        nc.gpsimd.dma_start(gi_sb[32*j:32*j+32],
                                      ag_out[t // CH, Hh*4+j, t % CH].rearrange("b (c n) -> b c n", c=3))
                    for k in range(KT):
                        lhsT = hT[k // 8][:, 32*(k % 8):32*(k % 8)+32]
                        for j in range(4):
                            base = (Hh*4+j)*768
                            nc.tensor.matmul(out=pa[32*j:32*j+32, :], lhsT=lhsT,
                                rhs=whh[:, k, base:base+512],
                                start=(k == 0), stop=(k == KT-1), tile_position=(0, 32*j))
                        for j in range(4):
                            base = (Hh*4+j)*768
                            mm_noldw(nc.tensor.matmul(out=pn[32*j:32*j+32, :], lhsT=lhsT,
                                rhs=whh[:, k, base+512:base+768],
                                start=(k == 0), stop=(k == KT-1), tile_position=(0, 32*j)))
                    # gates: r/z from pa+gi01; u=1-z and zh=z*hg precomputed off-path
                    ri = sb.tile([128, 512], f16, name=f"ri{t}{Hh}", tag="gt", bufs=1)
                    nc.vector.tensor_add(ri[:], pa[:], gi_sb[:, 0:2, :].rearrange("p c n -> p (c n)"))
                    r = sb.tile([128, 256], f16, name=f"r{t}{Hh}", tag="r", bufs=1)
                    nc.scalar.activation(r[:], ri[:, 0:256], AF.Sigmoid)
                    z = sb.tile([128, 256], f16, name=f"z{t}{Hh}", tag="z", bufs=1)
                    nc.scalar.activation(z[:], ri[:, 256:512], AF.Sigmoid)
                    u = sb.tile([128, 256], f16, name=f"u{t}{Hh}", tag="u", bufs=1)
                    nc.scalar.activation(u[:], z[:], AF.Identity, bias=1.0, scale=-1.0)
                    zh = sb.tile([128, 256], f16, name=f"zh{t}{Hh}", tag="zh", bufs=1)
                    nc.vector.tensor_mul(zh[:], z[:], hg[Hh][:])
                    # n path
                    v1 = sb.tile([128, 256], f16, name=f"v{t}{Hh}", tag="gt2", bufs=2)
                    nc.vector.tensor_add(v1[:], pn[:], bhn[:, Hh, :])
                    tn = sb.tile([128, 256], f16, name=f"tn{t}{Hh}", tag="gt2", bufs=2)
                    nc.vector.tensor_mul(tn[:], r[:], v1[:])
                    tn2 = sb.tile([128, 256], f16, name=f"tn2{t}{Hh}", tag="gt2", bufs=2)
                    nc.vector.tensor_add(tn2[:], tn[:], gi_sb[:, 2, :])
                    n_ = sb.tile([128, 256], f16, name=f"n{t}{Hh}", tag="n", bufs=1)
                    nc.scalar.activation(n_[:], tn2[:], AF.Tanh)
                    un = sb.tile([128, 256], f16, name=f"un{t}{Hh}", tag="un", bufs=1)
                    nc.vector.tensor_mul(un[:], u[:], n_[:])
                    hn = hp.tile([128, 256], f16, name=f"hg{Hh}", tag=f"hg{Hh}")
                    nc.vector.tensor_add(hn[:], un[:], zh[:])
                    newhg[Hh] = hn
                    nhT = hp.tile([128, 256], f16, name=f"hT{Hh}", tag=f"hT{Hh}")
                    nc.vector.transpose(nhT[:], hn[:])
                    newhT[Hh] = nhT
                    if store_hT0:
                        nc.sync.dma_start(out=hT0_seq[t, Hh], in_=nhT[:])
                    if store_hnat:
                        for j in range(4):
                            nc.sync.dma_start(
                                out=hnat[t, :, Hh*1024:(Hh+1)*1024]
                                    .rearrange("b (m j nl) -> b m j nl", m=8, j=4)[:, :, j, :],
                                in_=hn[32*j:32*j+32].rearrange("p (m nl) -> p m nl", m=8))
                hT = newhT
                hg = newhg

        # ================= run pipeline =================
        phase_x(wih0_d, b0_d, ag0_in, ag0_out, True, "a")
        recurrence(whh0_d, ag0_out, bhn0_d, True, False, "0")
        phase_x(wih1_d, b1_d, ag1_in, ag1_out, False, "c")
        recurrence(whh1_d, ag1_out, bhn1_d, False, True, "1")

        # ================= downstream workspace =================
        ds = big_tile("ds", 57344)  # (128, 28x2048) f16 in the big slot
        def R(i, w=2048):
            return ds[:, 2048*i: 2048*i + w]

        # ---- emb gather ----
        selt = sb.tile([128, SH], f16, name="selt", tag="selt")
        nc.sync.dma_start(selt[:], sel_d[:, :])
        hflat = hnat.ap().rearrange("t b f -> t (b f)")
        eflat = emb_mine.ap().rearrange("s b f -> s (b f)")
        for ch in range(16):
            reg = R([0, 2, 24][ch % 3], 4096)
            nc.sync.dma_start(reg, hflat[:, 4096*ch:4096*ch+4096])
            emc = R(4 + 2 * (ch % 2), 4096)[0:SH, :]
            for q in range(8):
                pse = ps_tile([SH, 512], f"pse{ch}{q}")
                nc.tensor.matmul(out=pse[:], lhsT=selt[:],
                                 rhs=reg[:, 512*q:512*q+512], start=True, stop=True)
                nc.scalar.activation(emc[:, 512*q:512*q+512], pse[:], AF.Identity)
            nc.scalar.dma_start(out=eflat[:, 4096*ch:4096*ch+4096], in_=emc)
            if DEBUG:
                nc.scalar.dma_start(
                    out=dbg_emb.ap().rearrange("s b f -> s (b f)")[:, 4096*ch:4096*ch+4096],
                    in_=emc)

        # ---- attention ----
        awt = sb.tile([128, 32], f16, name="awt", tag="awt")
        nc.sync.dma_start(awt[:], awt_d[:, :])
        ab = sb.tile([128, 1], f32, name="ab", tag="ab")
        nc.sync.dma_start(ab[:], ab_d[:, :])
        od = sb.tile([128, 4], f16, name="od", tag="od")
        nc.sync.dma_start(od[:], od_d[:, :])
        od2 = sb.tile([4, 128], f16, name="od2", tag="od2")
        nc.sync.dma_start(od2[:], od2_d[:, :])
        vec16 = R(4)[0:16, :]            # (16, 2048) f16
        for g in range(4):
            Ast16 = R(5 + (g % 2))       # tanh(emb) f16 (128, 2048)
            nc.sync.dma_start(R(7), emb_mine[4*g:4*g+4].rearrange("s b f -> (s b) f"))
            nc.scalar.activation(Ast16[:], R(7), AF.Tanh)
            EW = R(8 + (g % 2))
            psdP = ps_tile([128, 512], f"psdP{g}")
            nc.vector.memset(psdP[:], 1.0)
            psvP = ps_tile([128, 512], f"psvP{g}")
            for q in range(4):
                psaw = ps_tile([128, 512], f"psaw{g}{q}")
                for smp in range(4):
                    nc.tensor.matmul(
                        out=psaw[32*smp:32*smp+32, :],
                        lhsT=awt[32*smp:32*smp+32, :],
                        rhs=Ast16[32*smp:32*smp+32, 512*q:512*q+512],
                        start=True, stop=True, tile_position=(32*smp, 32*smp))
                nc.scalar.activation(EW[:, 512*q:512*q+512], psaw[:], AF.Exp,
                                     bias=ab[:, 0:1], scale=1.0)
                nc.tensor.matmul(out=psdP[32*q:32*q+4, :], lhsT=od[:],
                                 rhs=EW[:, 512*q:512*q+512],
                                 start=True, stop=True, tile_position=(0, 32*q),
                                 skip_group_check=True)
                V = R(12)[:, 512*q:512*q+512]
                nc.vector.tensor_mul(V, EW[:, 512*q:512*q+512], Ast16[:, 512*q:512*q+512])
                nc.tensor.matmul(out=psvP[32*q:32*q+4, :], lhsT=od[:], rhs=V,
                                 start=True, stop=True, tile_position=(0, 32*q),
                                 skip_group_check=True)
            rdenP = R(10)[:, 512*(g%2):512*(g%2)+512]
            with nc.allow_low_precision(reason="softmax recip fp16 ok"):
                nc.vector.reciprocal(rdenP, psdP[:])
            vtmpP = R(13)[:, 512*(g%2):512*(g%2)+512]
            nc.vector.tensor_mul(vtmpP, psvP[:], rdenP)
            for q in range(4):
                nc.sync.dma_start(out=vec16[4*g:4*g+4, 512*q:512*q+512],
                                  in_=vtmpP[32*q:32*q+4, :])
        attv = R(13)[0:16, :]
        nc.scalar.activation(attv, vec16, AF.Tanh)
        if DEBUG:
            nc.sync.dma_start(out=dbg_attv[:, :], in_=attv)

        # ---- build xnT (transposed features+ones) and xn_st ----
        zpad = sb.tile([128, 64], f16, name="zpad", tag="zpad")
        nc.vector.memset(zpad[:], 0.0)
        nc.vector.memset(zpad[:, 0:1], 1.0)
        for gg in range(4):
            nc.gpsimd.dma_start(out=att_pad[128*gg:128*gg+128, 64:128],
                              in_=zpad[:])
        for s in range(16):
            nc.gpsimd.dma_start(out=att_pad[32*s:32*s+32, 0:64],
                                in_=attv[s:s+1, :].rearrange("p (d h) -> p d h", d=32))
        xnT = R(14)[:, 0:512]
        nc.sync.dma_start_transpose(xnT, att_pad[:, :])
        xn_st = [R(14)[:, 512 + 64*g: 512 + 64*(g+1)] for g in range(4)]
        for g in range(4):
            for smp in range(4):
                nc.gpsimd.dma_start(out=xn_st[g][32*smp:32*smp+32, :],
                                  in_=attv[4*g+smp:4*g+smp+1, :].rearrange("p (d h) -> p d h", d=32))

        # ---- GAT ----
        gatw = sb.tile([65, 4, 64], f16, name="gatw", tag="gatw")
        nc.sync.dma_start(gatw[:], gw_d[:, :, :])
        gatt = sb.tile([128, 2, 64], f16, name="gatt", tag="gatt")
        nc.sync.dma_start(gatt[:], gatt_d.ap().rearrange("l p h -> p l h"))
        gbias = sb.tile([128, 2, 64], f16, name="gbias", tag="gbias")
        nc.sync.dma_start(gbias[:], gbias_d.ap().rearrange("l p h -> p l h"))

        def gat_layer(L, xT_all, gout_off):
            """xT_all (128, 512) f16 [rows 0:65 = features+ones].
            writes tanh(gat(x)) to R(gout_off)[:, 64g:64g+64] per g."""
            for g in range(4):
                psx = ps_tile([128, 128], f"psx{L}{g}")
                for smp in range(4):
                    bs = 4*g + smp
                    for lr in range(2):
                        nc.tensor.matmul(out=psx[32*smp:32*smp+32, 64*lr:64*lr+64],
                                         lhsT=xT_all[0:65, 32*bs:32*bs+32],
                                         rhs=gatw[:, 2*L+lr, :], start=True, stop=True,
                                         tile_position=(0, 32*smp))
                xl = R(15)[:, 128*g:128*g+64]
                nc.vector.tensor_copy(xl, psx[:, 0:64])
                xr = R(15)[:, 128*g+64:128*g+128]
                nc.vector.tensor_copy(xr, psx[:, 64:128])
                xrf = R(16)[0:4, :]
                for smp in range(4):
                    nc.gpsimd.dma_start(out=xrf[smp:smp+1, :].rearrange("p (d h) -> p d h", d=32),
                                      in_=xr[32*smp:32*smp+32, :])
                e3 = R(17 + g % 2)
                for q in range(4):
                    psxb = ps_tile([128, 512], f"psxb{L}{g}{q}")
                    nc.tensor.matmul(out=psxb[:], lhsT=od2[:], rhs=xrf[:, 512*q:512*q+512],
                                     start=True, stop=True)
                    e1 = R(19)[:, 0:512]
                    nc.vector.tensor_add(
                        e1.rearrange("p (d h) -> p d h", d=8), psxb[:].rearrange("p (d h) -> p d h", d=8),
                        xl[:, None, :].broadcast_to([128, 8, 64]))
                    e2 = R(19)[:, 512:1024]
                    nc.scalar.activation(e2, e1, AF.Lrelu, alpha=0.2)
                    nc.vector.tensor_mul(
                        e3[:, 512*q:512*q+512].rearrange("p (d h) -> p d h", d=8),
                        e2.rearrange("p (d h) -> p d h", d=8),
                        gatt[:, L, :][:, None, :].broadcast_to([128, 8, 64]))
                lg = sb.tile([128, 32], f32, name=f"lg{L}{g}", tag="lg", bufs=1)
                nc.vector.tensor_reduce(lg[:], e3[:].rearrange("p (d h) -> p d h", d=32),
                                        axis=mybir.AxisListType.X, op=mybir.AluOpType.add)
                elg = sb.tile([128, 32], f16, name=f"elg{L}{g}", tag="elg", bufs=1)
                nc.scalar.activation(elg[:], lg[:], AF.Exp)
                psd2 = ps_tile([4, 32], f"psd2{L}{g}")
                nc.tensor.matmul(out=psd2[:], lhsT=od[:], rhs=elg[:], start=True, stop=True)
                rd2 = sb.tile([4, 32], f16, name=f"rd2{L}{g}", tag="rd2", bufs=1)
                with nc.allow_low_precision(reason="softmax recip fp16 ok"):
                    nc.vector.reciprocal(rd2[:], psd2[:])
                psb2 = ps_tile([128, 32], f"psb2{L}{g}")
                nc.tensor.matmul(out=psb2[:], lhsT=od2[:], rhs=rd2[:], start=True, stop=True)
                alp = sb.tile([128, 32], f16, name=f"alp{L}{g}", tag="alp", bufs=1)
                nc.vector.tensor_mul(alp[:], elg[:], psb2[:])
                psg = ps_tile([128, 64], f"psg{L}{g}")
                for smp in range(4):
                    nc.tensor.matmul(out=psg[32*smp:32*smp+32, :],
                                     lhsT=alp[32*smp:32*smp+32, :],
                                     rhs=xl[32*smp:32*smp+32, :],
                                     start=True, stop=True,
                                     tile_position=(32*smp, 32*smp))
                gb = sb.tile([128, 64], f32, name=f"gb{L}{g}", tag="gb", bufs=1)
                nc.vector.tensor_add(gb[:], psg[:], gbias[:, L, :])
                nc.scalar.activation(R(gout_off)[:, 64*g:64*g+64], gb[:], AF.Tanh)

        gat_layer(0, xnT, 20)
        for gg in range(4):
            nc.gpsimd.dma_start(out=att_pad[128*gg:128*gg+128, 64:128], in_=zpad[:])
            nc.gpsimd.dma_start(out=att_pad[128*gg:128*gg+128, 0:64],
                                in_=R(20)[:, 64*gg:64*gg+64])
        g0T = R(21)[:, 0:512]
        nc.sync.dma_start_transpose(g0T, att_pad[:, :])
        gat_layer(1, g0T, 22)
        if DEBUG:
            nc.sync.dma_start(out=dbg_g01[0], in_=R(20)[:, 0:64])
            nc.sync.dma_start(out=dbg_g01[1], in_=R(22)[:, 0:64])

        # ---- fusion ----
        for g in range(4):
            gs = R(21)[:, 512 + 64*g: 512 + 64*(g+1)]
            nc.vector.tensor_add(gs, R(20)[:, 64*g:64*g+64], R(22)[:, 64*g:64*g+64])
            nc.gpsimd.dma_start(out=fus_nat[128*g:128*g+128, 0:64], in_=xn_st[g])
            nc.gpsimd.dma_start(out=fus_nat[128*g:128*g+128, 64:128], in_=gs)
        fusT = R(23)[:, 0:512]
        nc.sync.dma_start_transpose(fusT, fus_nat[:, :])

        # ---- caps (double-buffered workspaces A/B across mt) ----
        fwt = sb.tile([65, 32], f16, name="fwt", tag="fwt")
        nc.sync.dma_start(fwt[:], fw_d[:, :])
        for gg in range(4):
            nc.gpsimd.dma_start(out=caps_pad[128*gg:128*gg+128, 64:128], in_=zpad[:])
        od2c = R(27)[:, 0:512].rearrange("p (m c) -> p m c", m=4)
        nc.sync.dma_start(od2c[:], od2c_d.ap().rearrange("m p c -> p m c"))
        RA = [24, 16, 18, 20]
        RB = [25, 17, 19, 21]
        for mtg in range(4):
            o0s_l = [None]*4
            Lcur_l = [None]*4
            out_l = [None]*4
            for m in range(4):
                mt = 4*mtg + m
                wc = R(26)[:, 128*(mt % 8):128*(mt % 8)+128]
                nc.sync.dma_start(wc, wc_d[mt])
                pscap = ps_tile([128, 512], f"pscap{mt}")
                nc.tensor.matmul(out=pscap[:], lhsT=wc, rhs=fusT, start=True, stop=True)
                P = R(RA[m])[:, 0:512]
                nc.vector.tensor_copy(P, pscap[:])
                o0 = sb.tile([128, 16], f32, name=f"o0{mt}", tag="o0", bufs=2)
                nc.vector.tensor_reduce(o0[:], P.rearrange("p (b c) -> p b c", b=16),
                                        axis=mybir.AxisListType.X, op=mybir.AluOpType.add)
                o0s = R(RB[m])[:, 1536:1552]
                nc.vector.tensor_scalar_mul(o0s, o0[:], 1.0/32.0)
                Lcur = R(RA[m])[:, 512:1024]
                nc.vector.tensor_mul(Lcur.rearrange("p (b c) -> p b c", b=16),
                                     P.rearrange("p (b c) -> p b c", b=16),
                                     o0s[:, :, None].broadcast_to([128, 16, 32]))
                out_l[m] = o0s
                Lcur_l[m] = Lcur
            for it in (1, 2):
                psdC = ps_tile([128, 512], f"psdC{mtg}{it}")
                nc.vector.memset(psdC[:], 1.0)
                for m in range(4):
                    Et = R(RA[m])[:, 1024:1536]
                    nc.scalar.activation(Et, Lcur_l[m], AF.Exp)
                    nc.tensor.matmul(out=psdC[32*m:32*m+4, :], lhsT=od[:], rhs=Et,
                                     start=True, stop=True, tile_position=(0, 32*m),
                                     skip_group_check=True)
                rdenC = R(22)[:, 0:512]
                with nc.allow_low_precision(reason="softmax recip fp16 ok"):
                    nc.vector.reciprocal(rdenC, psdC[:])
                for m in range(4):
                    mt = 4*mtg + m
                    P = R(RA[m])[:, 0:512]
                    Et = R(RA[m])[:, 1024:1536]
                    psbc = ps_tile([128, 512], f"psbc{mt}{it}")
                    nc.tensor.matmul(out=psbc[:], lhsT=od2c[:, m, :], rhs=rdenC,
                                     start=True, stop=True)
                    pt = R(RA[m])[:, 1536:2048]
                    nc.vector.tensor_mul(pt, Et, psbc[:])
                    pp = R(RB[m])[:, 0:512]
                    nc.vector.tensor_mul(pp, pt, P)
                    oo = sb.tile([128, 16], f32, name=f"oo{mt}{it}", tag="o0", bufs=2)
                    nc.vector.tensor_reduce(oo[:], pp.rearrange("p (b c) -> p b c", b=16),
                                            axis=mybir.AxisListType.X, op=mybir.AluOpType.add)
                    oos = R(RB[m])[:, 1552 + 16*it: 1568 + 16*it]
                    nc.vector.tensor_copy(oos, oo[:])
                    out_l[m] = oos
                    if it == 1:
                        m2 = R(RB[m])[:, 512:1024]
                        nc.vector.tensor_mul(m2.rearrange("p (b c) -> p b c", b=16),
                                             P.rearrange("p (b c) -> p b c", b=16),
                                             oos[:, :, None].broadcast_to([128, 16, 32]))
                        L2 = R(RB[m])[:, 1024:1536]
                        nc.vector.tensor_add(L2, Lcur_l[m], m2)
                        Lcur_l[m] = L2
            for m in range(4):
                mt = 4*mtg + m
                tc_t = R(RB[m])[:, 1600:1616]
                nc.scalar.activation(tc_t, out_l[m], AF.Tanh)
                if DEBUG:
                    nc.sync.dma_start(out=dbg_caps[mt], in_=tc_t)
                for l_loc in range(4):
                    nc.gpsimd.dma_start(
                        out=caps_pad[:, 4*mt+l_loc].rearrange("(s o) -> o s", s=16),
                        in_=tc_t[32*l_loc:32*l_loc+32, :])
        capsT = R(23)[:, 512:1024]
        nc.sync.dma_start_transpose(capsT, caps_pad[:, :])
        psf = ps_tile([32, 512], "psf")
        nc.tensor.matmul(out=psf[:], lhsT=fwt[:], rhs=capsT[0:65, :], start=True, stop=True)
        fin = R(25)[0:32, 1024:1536]
        nc.scalar.activation(fin, psf[:], AF.Tanh)
        nc.sync.dma_start(out=out_d.ap().rearrange("dd s o -> dd (s o)"),
                          in_=fin)

        for p_ in reversed(ctxs):
            p_.__exit__(None, None, None)
    nc.compile()
    return nc


# ===================== host side =====================
_NC_CACHE = {}

def _get_program():
    if "prog" not in _NC_CACHE:
        _NC_CACHE["prog"] = build_program()
    return _NC_CACHE["prog"]


def _prep_inputs(inputs):
    X = np.asarray(inputs["inputs"], np.float32)
    X = np.nan_to_num(X, nan=0.0, posinf=1.0)
    ei = np.asarray(inputs["edge_index"])
    s = np.repeat(np.arange(D), D); t = np.tile(np.arange(D), D)
    off = (np.arange(B) * D)[:, None]
    exp_ei = np.stack([(s[None] + off).reshape(-1), (t[None] + off).reshape(-1)]).astype(ei.dtype)
    assert np.array_equal(ei, exp_ei), "edge_index mismatch vs block-diagonal pattern"

    # [B, 128, KT*32]: xT2[b, p, k*32+c] = X[b, c, k*128+p]
    xT = np.ascontiguousarray(
        np.swapaxes(X, 1, 2).reshape(B, KT, 128, 32).transpose(0, 2, 1, 3)
        .reshape(B, 128, KT*32)).astype(np.float16)

    wih0p = _gate_cols(np.asarray(inputs["Wih0"], np.float32))
    wih1p = _gate_cols(np.asarray(inputs["Wih1"], np.float32))
    whh0p = _gate_cols(np.asarray(inputs["Whh0"], np.float32))
    whh1p = _gate_cols(np.asarray(inputs["Whh1"], np.float32))
    whh0_dev = np.ascontiguousarray(whh0p.reshape(KT, 128, 6144)).astype(np.float16)
    whh1_dev = np.ascontiguousarray(whh1p.reshape(KT, 128, 6144)).astype(np.float16)

    def bias_strip(bih, bhh):
        b = np.zeros(6144, np.float32)
        for q in range(8):
            hds = PERM[q*256:(q+1)*256]
            b[q*768+0*256: q*768+1*256] = bih[0*2048 + hds] + bhh[0*2048 + hds]
            b[q*768+1*256: q*768+2*256] = bih[1*2048 + hds] + bhh[1*2048 + hds]
            b[q*768+2*256: q*768+3*256] = bih[2*2048 + hds]
        return b
    bih0 = np.asarray(inputs["bih0"], np.float32); bhh0 = np.asarray(inputs["bhh0"], np.float32)
    bih1 = np.asarray(inputs["bih1"], np.float32); bhh1 = np.asarray(inputs["bhh1"], np.float32)
    bs0 = bias_strip(bih0, bhh0).astype(np.float16)
    bs1 = bias_strip(bih1, bhh1).astype(np.float16)

    def bhn_bcast(bhh):
        outb = np.zeros((128, 2, 256), np.float32)
        for Hh in range(2):
            for j in range(4):
                hds = PERM[(Hh*4+j)*256:(Hh*4+j)*256+256]
                outb[32*j:32*j+32, Hh, :] = bhh[2*2048 + hds][None, :]
        return outb.astype(np.float16)

    A_w = np.asarray(inputs["A_w"], np.float32); A_b = np.asarray(inputs["A_b"], np.float32)
    awt = np.tile(A_w.T.astype(np.float16), (4, 1))
    ab = np.tile(A_b, 4)[:, None].astype(np.float32)
    od = np.zeros((128, 4), np.float16)
    for gq in range(4):
        od[32*gq:32*gq+32, gq] = 1.0
    od2 = np.ascontiguousarray(od.T)

    gw = np.zeros((65, 4, 64), np.float16)
    for L, pfx in enumerate(["g0", "g1"]):
        for lr, nm in enumerate(["l", "r"]):
            gw[0:64, 2*L+lr] = np.asarray(inputs[f"{pfx}_W{nm}"], np.float32).T.astype(np.float16)
            gw[64, 2*L+lr] = np.asarray(inputs[f"{pfx}_b{nm}"], np.float32).astype(np.float16)
    gat_att = np.zeros((2, 128, 64), np.float16)
    gat_bias = np.zeros((2, 128, 64), np.float16)
    for L, pfx in enumerate(["g0", "g1"]):
        gat_att[L] = np.tile(np.asarray(inputs[f"{pfx}_att"], np.float32), (128, 1)).astype(np.float16)
        gat_bias[L] = np.tile(np.asarray(inputs[f"{pfx}_bias"], np.float32), (128, 1)).astype(np.float16)

    Wc = np.asarray(inputs["W_caps"], np.float32)
    wc_t = np.zeros((16, 128, 128), np.float16)
    for mt in range(16):
        for l_loc in range(4):
            l = 4*mt + l_loc
            wc_t[mt, :, 32*l_loc:32*l_loc+32] = Wc[:, l, :].T.astype(np.float16)
    od2c = np.zeros((4, 128, 128), np.float16)
    for m in range(4):
        for g in range(4):
            for t_ in range(32):
                od2c[m, 32*m+g, 32*g+t_] = 1.0
    fw = np.zeros((65, 32), np.float16)
    fw[0:64] = np.asarray(inputs["F_w"], np.float32).T.astype(np.float16)
    fw[64] = np.asarray(inputs["F_b"], np.float32).astype(np.float16)

    common = dict(xT=xT, whh0=whh0_dev, whh1=whh1_dev,
                  bhn0=bhn_bcast(bhh0), bhn1=bhn_bcast(bhh1),
                  awt=awt, ab=ab, od=od, od2=od2, gw=gw, gat_att=gat_att,
                  gat_bias=gat_bias, wc=wc_t, fw=fw, od2c=od2c)
    in_maps = []
    for r in range(NC):
        sel = np.zeros((128, SH), np.float16)
        for i in range(SH):
            sel[SH*r + i, i] = 1.0
        m = dict(common)
        m["wih0"] = np.ascontiguousarray(wih0p[:, 768*r:768*r+768].astype(np.float16).reshape(KT, 128, 768))
        m["wih1"] = np.ascontiguousarray(wih1p[:, 768*r:768*r+768].astype(np.float16).reshape(KT, 128, 768))
        m["b0"] = bs0[768*r:768*r+768][None, :].copy()
        m["b1"] = bs1[768*r:768*r+768][None, :].copy()
        m["sel"] = sel
        in_maps.append(m)
    return in_maps


def kernel(**inputs):
    in_maps = _prep_inputs(inputs)
    nc = _get_program()
    res = run_bass_kernel_spmd(nc, in_maps, list(range(NC)))
    out = np.concatenate([res.results[r]["out"].transpose(1, 2, 0) for r in range(NC)], axis=0)
    return out.astype(np.float32)


if __name__ == "__main__":
    t0 = time.time()
    build_program()
    print("build+compile", time.time() - t0)
